# revision 1
# baseline (speedup 1.0000x reference)
"""Trainium2 Bass kernel for 3-layer GAT (nn_MultiLayerGAT).

Strategy (dst-node sharding, 8 cores):
  - Add self-loops, sort edges by dst. Nodes padded to 10240 = 80 blocks of
    128; core k owns blocks [10k, 10k+10). Each block's edge list is padded
    to a uniform CK chunks of 128 edges (same CK for all cores => one SPMD
    program).
  - Per layer:
    Phase A (replicated): xp_ext = h @ W_ext -> HBM rows
        [xp(256) | al_s(8) | pad | al_d(8) | pad] (384 f32; layer3: 64 f32)
      where W_ext = [W | W@a_src_blockdiag | W@a_dst_blockdiag] so the
      attention logits come out of the same matmul.
    Phase B (sharded): per dst block: dma_gather xp_ext rows by src
      (attention sources) + a small dma_gather of al_d windows by dst;
      e = lrelu(al_s[src]+al_d[dst]); ee = exp(e) (no max subtraction --
      validated safe, e in [-1.1, 5.6]); scale gathered features by ee
      (step-0 broadcast); segment-sum via one-hot matmul on TensorE
      (fp32r): out_psum[128 dst, 264] += onehot(dstcode)^T @ [ee*G | ee].
      Divide by the summed ee column, add bias, ELU, transpose (layers
      1-2) for the next layer's lhsT.
    Phase C (layers 1-2): AllGather of transposed h shards so every core
      has the full h^T for the next layer's Phase A.
  - Final layer: single head, out = log_softmax(out_pre/s + b3) per block.

Pads: gather idx 0 (finite garbage), dstcode -1 => one-hot column is all
zero, so pads contribute nothing to either numerator or denominator.
"""

import numpy as np

N = 10000
E = 320000
IN = 128
HID = 32
HEADS = 8
HC = HEADS * HID          # 256
OUT = 40
NEG = 0.2

NPAD = 10240              # 80 blocks of 128
NBLK_TOT = NPAD // 128    # 80
NCORES = 8
NB = NBLK_TOT // NCORES   # 10 blocks per core

ROW12 = 384               # xp_ext row floats, layers 1-2
ROW3 = 64                 # layer 3


# ----------------------------------------------------------------------------
# host-side preprocessing
# ----------------------------------------------------------------------------

def build_w_ext(W, a_src, a_dst, row):
    """W_ext[in, row]: [W | W@As | pad | W@Ad | pad] with As/Ad block-diag."""
    inn, hc = W.shape
    H, C = a_src.shape
    As = np.zeros((hc, H), np.float32)
    Ad = np.zeros((hc, H), np.float32)
    for h in range(H):
        As[h * C:(h + 1) * C, h] = a_src[h]
        Ad[h * C:(h + 1) * C, h] = a_dst[h]
    We = np.zeros((inn, row), np.float32)
    We[:, 0:hc] = W
    if row == ROW12:
        We[:, 256:264] = W @ As
        We[:, 320:328] = W @ Ad
    else:  # layer 3: [xp(40) | al_s(1) | al_d(1) | pad]
        We[:, 40:41] = W @ As
        We[:, 41:42] = W @ Ad
    return We


def preprocess(edge_index):
    """Chunk tables shared by all layers. Returns (CK, per-core arrays)."""
    src = np.concatenate([edge_index[0], np.arange(N, dtype=edge_index.dtype)])
    dst = np.concatenate([edge_index[1], np.arange(N, dtype=edge_index.dtype)])
    src = src.astype(np.int64)
    dst = dst.astype(np.int64)
    order = np.argsort(dst, kind="stable")
    ssrc, sdst = src[order], dst[order]
    blk = sdst // 128                                  # block of each edge
    # edges per global block
    cnt = np.bincount(blk, minlength=NBLK_TOT)
    CK = int(np.ceil(cnt.max() / 128))
    S = CK * 128                                       # slots per block
    starts = np.concatenate([[0], np.cumsum(cnt)])

    gsrc = np.zeros((NBLK_TOT, S), np.int64)           # gather idx (by src)
    gdst = np.zeros((NBLK_TOT, S), np.int64)           # gather idx (by dst)
    dstc = np.full((NBLK_TOT, S), -1.0, np.float32)    # dst - 128*block
    for b in range(NBLK_TOT):
        lo, hi = starts[b], starts[b + 1]
        n = hi - lo
        gsrc[b, :n] = ssrc[lo:hi]
        gdst[b, :n] = sdst[lo:hi]
        dstc[b, :n] = (sdst[lo:hi] - 128 * b).astype(np.float32)

    def wrap16(idx_flat):
        # [S] -> [128, S//16] int16 tile; idx i -> [i%16, i//16], and the
        # 16-partition pattern replicated to all 128 partitions (each GPSIMD
        # Q7 core reads its own partition group on hardware)
        t16 = idx_flat.reshape(S // 16, 16).T.astype(np.int16)
        return np.tile(t16, (8, 1))

    cores = []
    for k in range(NCORES):
        bsl = slice(k * NB, (k + 1) * NB)
        gsrc_t = np.concatenate([wrap16(gsrc[b]) for b in range(k * NB, (k + 1) * NB)], axis=1)
        gdst_t = np.concatenate([wrap16(gdst[b]) for b in range(k * NB, (k + 1) * NB)], axis=1)
        # dstcode per chunk column: [128, NB*CK], col (b*CK+j)[p] = code of edge j*128+p
        dc = dstc[bsl].reshape(NB, CK, 128).transpose(2, 0, 1).reshape(128, NB * CK)
        cores.append(dict(gsrc=gsrc_t, gdst=gdst_t, dstc=np.ascontiguousarray(dc)))
    return CK, cores


# ----------------------------------------------------------------------------
# bass program
# ----------------------------------------------------------------------------

def build_nc(CK):
    import concourse.bacc as bacc
    import concourse.mybir as mybir
    import concourse.tile as tile
    from concourse.library_config import mlp

    f32 = mybir.dt.float32
    f32r = mybir.dt.float32r
    i16 = mybir.dt.int16
    Alu = mybir.AluOpType
    Act = mybir.ActivationFunctionType

    S = CK * 128

    nc = bacc.Bacc("TRN2", debug=False)

    # inputs (per core)
    xT = nc.dram_tensor("xT", [IN, NPAD], f32, kind="ExternalInput")
    W1e = nc.dram_tensor("W1e", [IN, ROW12], f32, kind="ExternalInput")
    W2e = nc.dram_tensor("W2e", [HC, ROW12], f32, kind="ExternalInput")
    W3e = nc.dram_tensor("W3e", [HC, ROW3], f32, kind="ExternalInput")
    gsrc = nc.dram_tensor("gsrc", [128, NB * S // 16], i16, kind="ExternalInput")
    gdst = nc.dram_tensor("gdst", [128, NB * S // 16], i16, kind="ExternalInput")
    dstc = nc.dram_tensor("dstc", [128, NB * CK], f32, kind="ExternalInput")
    iota = nc.dram_tensor("iota", [128, 128], f32, kind="ExternalInput")
    ident = nc.dram_tensor("ident", [128, 128], f32, kind="ExternalInput")
    b1r = nc.dram_tensor("b1r", [128, HC], f32, kind="ExternalInput")
    b2r = nc.dram_tensor("b2r", [128, HC], f32, kind="ExternalInput")
    b3r = nc.dram_tensor("b3r", [128, OUT], f32, kind="ExternalInput")

    out = nc.dram_tensor("out", [NB * 128, OUT], f32, kind="ExternalOutput")

    import os
    debug = bool(int(os.environ.get("GAT_DEBUG", "0")))
    if debug:
        dbg_ee = nc.dram_tensor("dbg_ee", [128, CK * 8], f32, kind="ExternalOutput")
        dbg_als = nc.dram_tensor("dbg_als", [128, CK * 8], f32, kind="ExternalOutput")
        dbg_ald = nc.dram_tensor("dbg_ald", [128, CK * 8], f32, kind="ExternalOutput")
        dbg_ps = nc.dram_tensor("dbg_ps", [128, 264], f32, kind="ExternalOutput")

    # scratch DRAM
    xe12a = nc.dram_tensor("xe12a", [NPAD, ROW12], f32)
    xe12b = nc.dram_tensor("xe12b", [NPAD, ROW12], f32)
    xe3 = nc.dram_tensor("xe3", [NPAD, ROW3], f32)
    cc_in1 = nc.dram_tensor("cc_in1", [HC, NB * 128], f32)
    cc_out1 = nc.dram_tensor("cc_out1", [NCORES * HC, NB * 128], f32,
                             addr_space="Shared")
    cc_in2 = nc.dram_tensor("cc_in2", [HC, NB * 128], f32)
    cc_out2 = nc.dram_tensor("cc_out2", [NCORES * HC, NB * 128], f32,
                             addr_space="Shared")

    with tile.TileContext(nc) as tc:
        nc.gpsimd.load_library(mlp)
        with tc.tile_pool(name="const", bufs=1) as cpool, \
             tc.tile_pool(name="w", bufs=1) as wpool, \
             tc.tile_pool(name="lhs", bufs=3) as lhspool, \
             tc.tile_pool(name="xps", bufs=3) as xpspool, \
             tc.tile_pool(name="gath", bufs=2) as gpool, \
             tc.tile_pool(name="small", bufs=2) as spool, \
             tc.tile_pool(name="oh", bufs=4) as ohpool, \
             tc.tile_pool(name="post", bufs=2) as ppool, \
             tc.tile_pool(name="psA", bufs=2, space="PSUM") as psA, \
             tc.tile_pool(name="psB", bufs=2, space="PSUM") as psB, \
             tc.tile_pool(name="psT", bufs=2, space="PSUM") as psT:

            # constants resident in SBUF
            gsrc_t = cpool.tile([128, NB * S // 16], i16, tag="gsrc")
            nc.sync.dma_start(gsrc_t[:], gsrc[:])
            gdst_t = cpool.tile([128, NB * S // 16], i16, tag="gdst")
            nc.sync.dma_start(gdst_t[:], gdst[:])
            dstc_t = cpool.tile([128, NB * CK], f32, tag="dstc")
            nc.sync.dma_start(dstc_t[:], dstc[:])
            iota_t = cpool.tile([128, 128], f32, tag="iota")
            nc.sync.dma_start(iota_t[:], iota[:])
            ident_t = cpool.tile([128, 128], f32, tag="ident")
            nc.sync.dma_start(ident_t[:], ident[:])
            b1_t = cpool.tile([128, HC], f32, tag="b1")
            nc.sync.dma_start(b1_t[:], b1r[:])
            b2_t = cpool.tile([128, HC], f32, tag="b2")
            nc.sync.dma_start(b2_t[:], b2r[:])
            b3_t = cpool.tile([128, OUT], f32, tag="b3")
            nc.sync.dma_start(b3_t[:], b3r[:])

            def phase_a(layer, we_dram, row, xe_dram, cc_out_dram):
                """xp_ext = h @ W_ext for all NPAD nodes -> xe_dram."""
                kchunks = 1 if layer == 1 else 2
                w_t = wpool.tile([128, kchunks, row], f32, tag=f"w{layer}")
                for kk in range(kchunks):
                    nc.sync.dma_start(w_t[:, kk, :], we_dram[kk * 128:(kk + 1) * 128, :])
                for t in range(NBLK_TOT):
                    ps = psA.tile([128, row], f32, tag="xps")
                    for kk in range(kchunks):
                        lhs = lhspool.tile([128, 128], f32, tag="lhs")
                        if layer == 1:
                            nc.sync.dma_start(
                                lhs[:], xT[:, t * 128:(t + 1) * 128])
                        else:
                            g = (t // NB) * HC + kk * 128
                            c = (t % NB) * 128
                            nc.sync.dma_start(
                                lhs[:], cc_out_dram[g:g + 128, c:c + 128])
                        lhs = lhs[:]
                        nc.tensor.matmul(
                            ps[:], lhs, w_t[:, kk, :],
                            start=(kk == 0), stop=(kk == kchunks - 1))
                    sb = xpspool.tile([128, row], f32, tag="xpsb")
                    nc.vector.tensor_copy(sb[:], ps[:])
                    nc.sync.dma_start(xe_dram[t * 128:(t + 1) * 128, :], sb[:])

            def phase_b(layer, row, xe_dram, cc_in_dram, b_t):
                """aggregation over this core's NB blocks."""
                nh = HEADS if layer < 3 else 1
                fe = HC if layer < 3 else OUT          # feature width
                rw = fe + nh                           # matmul rhs width
                alo = fe                               # al_s offset in row
                grow = 320 if layer < 3 else ROW3      # main gather elem
                adw = 64 if layer < 3 else ROW3        # al_d gather elem
                adc = 0 if layer < 3 else 41           # al_d col within window
                for b in range(NB):
                    g_t = gpool.tile([128, CK, grow], f32, tag="G")
                    isl = gsrc_t[:, b * S // 16:(b + 1) * S // 16]
                    nc.gpsimd.dma_gather(
                        g_t[:], xe_dram[:, 0:grow], isl, S, S, grow,
                        elem_step=row, single_packet=False)
                    ad_t = gpool.tile([128, CK, adw], f32, tag="AD")
                    dsl = gdst_t[:, b * S // 16:(b + 1) * S // 16]
                    nc.gpsimd.dma_gather(
                        ad_t[:], xe_dram[:, (0 if layer == 3 else 320):row],
                        dsl, S, S, adw, elem_step=row, single_packet=False)

                    # e = lrelu(al_s + al_d); ee = exp(e)
                    ee_t = spool.tile([128, CK, nh], f32, tag="ee")
                    if debug and layer == 1 and b == 0:
                        dtmp = ppool.tile([128, CK * nh], f32, tag="dbg")
                        nc.vector.tensor_copy(
                            dtmp[:].rearrange("p (c h) -> p c h", h=nh),
                            g_t[:, :, alo:alo + nh])
                        nc.sync.dma_start(dbg_als[:], dtmp[:])
                        dtmp2 = ppool.tile([128, CK * nh], f32, tag="dbg2")
                        nc.vector.tensor_copy(
                            dtmp2[:].rearrange("p (c h) -> p c h", h=nh),
                            ad_t[:, :, adc:adc + nh])
                        nc.sync.dma_start(dbg_ald[:], dtmp2[:])
                    nc.vector.tensor_tensor(
                        ee_t[:], g_t[:, :, alo:alo + nh],
                        ad_t[:, :, adc:adc + nh], Alu.add)
                    eef = ee_t[:].rearrange("p c h -> p (c h)")
                    nc.vector.scalar_tensor_tensor(
                        eef, eef, NEG, eef, Alu.mult, Alu.max)
                    nc.scalar.activation(eef, eef, Act.Exp)
                    if debug and layer == 1 and b == 0:
                        nc.sync.dma_start(dbg_ee[:], eef)

                    # scale features in place, stash ee next to them
                    if layer < 3:
                        nc.vector.tensor_tensor(
                            g_t[:, :, 0:fe].rearrange("p c (h z) -> p c h z", z=HID),
                            g_t[:, :, 0:fe].rearrange("p c (h z) -> p c h z", z=HID),
                            ee_t[:].to_broadcast([128, CK, nh, HID]),
                            Alu.mult)
                    else:
                        nc.vector.tensor_tensor(
                            g_t[:, :, 0:fe],
                            g_t[:, :, 0:fe],
                            ee_t[:].rearrange("p c h -> p (c h)").to_broadcast([128, CK, fe]),
                            Alu.mult)
                    nc.vector.tensor_copy(g_t[:, :, fe:fe + nh], ee_t[:])

                    # one-hot tiles and the segment matmul
                    ps = psB.tile([128, rw], f32, tag="agg")
                    for j in range(CK):
                        oh_t = ohpool.tile([128, 128], f32, tag="oh")
                        nc.vector.tensor_scalar(
                            oh_t[:], iota_t[:],
                            dstc_t[:, b * CK + j:b * CK + j + 1], None,
                            Alu.is_equal)
                        nc.tensor.matmul(
                            ps[:], oh_t[:], g_t[:, j, 0:rw],
                            start=(j == 0), stop=(j == CK - 1))

                    if debug and layer == 1 and b == 0:
                        dtmp3 = ppool.tile([128, 264], f32, tag="dbg3")
                        nc.vector.tensor_copy(dtmp3[:], ps[:])
                        nc.sync.dma_start(dbg_ps[:], dtmp3[:])

                    # divide by ee-sum, bias (+eps like the reference; also
                    # keeps pad dst rows, which have s=0, finite)
                    r_t = spool.tile([128, nh], f32, tag="recip")
                    nc.vector.tensor_scalar(
                        r_t[:], ps[:, fe:fe + nh], 1e-16, None, Alu.add)
                    nc.vector.reciprocal(r_t[:], r_t[:])
                    h_t = ppool.tile([128, fe], f32, tag="H")
                    if layer < 3:
                        nc.vector.tensor_tensor(
                            h_t[:].rearrange("p (h z) -> p h z", z=HID),
                            ps[:, 0:fe].rearrange("p (h z) -> p h z", z=HID),
                            r_t[:].to_broadcast([128, nh, HID]),
                            Alu.mult)
                    else:
                        nc.vector.tensor_scalar(
                            h_t[:], ps[:, 0:fe], r_t[:], None, Alu.mult)
                    nc.vector.tensor_tensor(h_t[:], h_t[:], b_t[:], Alu.add)

                    if layer < 3:
                        # ELU: relu(z) + exp(min(z,0)) - 1, then transpose
                        t2 = ppool.tile([128, fe], f32, tag="elu")
                        nc.vector.tensor_scalar(t2[:], h_t[:], 0.0, None, Alu.min)
                        nc.scalar.activation(t2[:], t2[:], Act.Exp)
                        nc.vector.scalar_tensor_tensor(
                            h_t[:], h_t[:], 0.0, t2[:], Alu.max, Alu.add)
                        nc.vector.tensor_scalar(h_t[:], h_t[:], -1.0, None, Alu.add)
                        for half in range(2):
                            pt = psT.tile([128, 128], f32, tag="tr")
                            nc.tensor.transpose(
                                pt[:], h_t[:, half * 128:(half + 1) * 128],
                                ident_t[:])
                            st = ppool.tile([128, 128], f32, tag="trs")
                            nc.vector.tensor_copy(st[:], pt[:])
                            nc.sync.dma_start(
                                cc_in_dram[half * 128:(half + 1) * 128,
                                           b * 128:(b + 1) * 128], st[:])
                    else:
                        # log_softmax over the 40 classes
                        m_t = spool.tile([128, 1], f32, tag="m")
                        nc.vector.tensor_reduce(
                            m_t[:], h_t[:], mybir.AxisListType.X, Alu.max)
                        nc.vector.tensor_scalar(
                            h_t[:], h_t[:], m_t[:], None, Alu.subtract)
                        x_t = ppool.tile([128, fe], f32, tag="exps")
                        s_t = spool.tile([128, 1], f32, tag="s")
                        nc.scalar.activation(
                            x_t[:], h_t[:], Act.Exp, accum_out=s_t[:])
                        l_t = spool.tile([128, 1], f32, tag="l")
                        nc.scalar.activation(l_t[:], s_t[:], Act.Ln)
                        nc.vector.tensor_scalar(
                            h_t[:], h_t[:], l_t[:], None, Alu.subtract)
                        nc.sync.dma_start(
                            out[b * 128:(b + 1) * 128, :], h_t[:])

            mode = os.environ.get("GAT_MODE", "full")
            if mode == "a":
                # phase A of layer 1 only; dump a slice of xe12a to out
                phase_a(1, W1e, ROW12, xe12a, None)
                for b in range(NB):
                    t_ = ppool.tile([128, OUT], f32, tag="dump")
                    nc.sync.dma_start(t_[:], xe12a[b * 128:(b + 1) * 128, 0:OUT])
                    nc.sync.dma_start(out[b * 128:(b + 1) * 128, :], t_[:])
            elif mode == "b":
                # layer 1 end-to-end without collective; dump h1 cols
                phase_a(1, W1e, ROW12, xe12a, None)
                phase_b(1, ROW12, xe12a, cc_in1, b1_t)
                for b in range(NB):
                    t_ = ppool.tile([128, OUT], f32, tag="dump")
                    nc.sync.dma_start(
                        t_[:], cc_in1[0:OUT, b * 128:(b + 1) * 128]
                        .rearrange("a b -> b a"))
                    nc.sync.dma_start(out[b * 128:(b + 1) * 128, :], t_[:])
            else:
                # layer 1
                phase_a(1, W1e, ROW12, xe12a, None)
                phase_b(1, ROW12, xe12a, cc_in1, b1_t)
                nc.gpsimd.collective_compute(
                    "AllGather", mybir.AluOpType.bypass,
                    replica_groups=[list(range(NCORES))],
                    ins=[cc_in1.ap().opt()], outs=[cc_out1.ap().opt()])
                # layer 2
                phase_a(2, W2e, ROW12, xe12b, cc_out1)
                phase_b(2, ROW12, xe12b, cc_in2, b2_t)
                nc.gpsimd.collective_compute(
                    "AllGather", mybir.AluOpType.bypass,
                    replica_groups=[list(range(NCORES))],
                    ins=[cc_in2.ap().opt()], outs=[cc_out2.ap().opt()])
                # layer 3
                phase_a(3, W3e, ROW3, xe3, cc_out2)
                phase_b(3, ROW3, xe3, None, b3_t)

    nc.compile()
    return nc


# ----------------------------------------------------------------------------
# entry point
# ----------------------------------------------------------------------------

LAST_EXEC_NS = None


def kernel(**inputs):
    import os
    from concourse.bass_utils import run_bass_kernel_spmd
    global LAST_EXEC_NS

    x = np.asarray(inputs["x"], np.float32)
    ei = np.asarray(inputs["edge_index"])
    CK, cores = preprocess(ei)

    xTn = np.zeros((IN, NPAD), np.float32)
    xTn[:, 0:N] = x.T
    W1en = build_w_ext(np.asarray(inputs["W1"], np.float32),
                       np.asarray(inputs["a_src1"], np.float32),
                       np.asarray(inputs["a_dst1"], np.float32), ROW12)
    W2en = build_w_ext(np.asarray(inputs["W2"], np.float32),
                       np.asarray(inputs["a_src2"], np.float32),
                       np.asarray(inputs["a_dst2"], np.float32), ROW12)
    W3en = build_w_ext(np.asarray(inputs["W3"], np.float32),
                       np.asarray(inputs["a_src3"], np.float32),
                       np.asarray(inputs["a_dst3"], np.float32), ROW3)
    iota_n = np.tile(np.arange(128, dtype=np.float32), (128, 1))
    ident_n = np.eye(128, dtype=np.float32)
    b1n = np.tile(np.asarray(inputs["b1"], np.float32), (128, 1))
    b2n = np.tile(np.asarray(inputs["b2"], np.float32), (128, 1))
    b3n = np.tile(np.asarray(inputs["b3"], np.float32), (128, 1))

    nc = build_nc(CK)
    in_maps = []
    for k in range(NCORES):
        in_maps.append({
            "xT": xTn, "W1e": W1en, "W2e": W2en, "W3e": W3en,
            "gsrc": cores[k]["gsrc"], "gdst": cores[k]["gdst"],
            "dstc": cores[k]["dstc"], "iota": iota_n, "ident": ident_n,
            "b1r": b1n, "b2r": b2n, "b3r": b3n,
        })
    trace = bool(int(os.environ.get("GAT_TRACE", "0")))
    res = run_bass_kernel_spmd(nc, in_maps, list(range(NCORES)), trace=trace)
    LAST_EXEC_NS = res.exec_time_ns
    full = np.concatenate([res.results[k]["out"] for k in range(NCORES)], axis=0)
    return full[0:N].astype(np.float32)



# revision 4
# speedup vs baseline: 2.4047x; 2.4047x over previous
"""Trainium2 Bass kernel for 3-layer GAT (nn_MultiLayerGAT), v2.

Strategy (dst-node sharding, 8 cores, fp16 data path):
  - Add self-loops, sort edges by dst. Nodes padded to 10240 = 80 blocks of
    128; core k owns blocks [10k, 10k+10). Each block's edges padded to CK
    chunks of 128 slots (same CK for all cores => one SPMD program).
  - Phase A is SHARDED: each core computes xp_ext = h @ W_ext for its own 10
    blocks only (lhsT = resident h^T produced by the previous layer's phase
    B), writes a [1280, row] fp16 shard, then an AllGather replicates the
    full [10240, row] xe to every core.  xe row = [xp | al_s | pad] fp16;
    al_d is kept in a tiny per-core resident SBUF tile (never shipped).
  - Phase B per dst block:
      one-hot(dstcode) tiles are built in ONE DVE is_equal op per block from
      host-precomputed code tables (dstc broadcast vs iota); the TRANSPOSED
      one-hot (for al_d expansion) is built from a host-replicated code table
      (dstcB) vs a per-partition iota.  al_d[dst] per edge = ohT^T @ al_d_blk
      on TensorE (no dma_gather for al_d at all).
      ONE gpsimd dma_gather per block fetches [xp|al_s] rows by src (768B
      elements, fp16).  Gathers rotate across SWDGE queues so descriptor
      generation runs concurrently on different Q7 core pairs (the 41us/4352
      idx desc-gen is otherwise the critical path).
      ee = exp(lrelu(al_s+al_d)); scale gathered xp by ee (broadcast mult);
      segment-sum via one-hot matmul on TensorE (fp16, fp32 PSUM accum):
      psum[128 dst, fe+nh] += oh^T @ [ee*G | ee].  Divide by the summed ee,
      bias, ELU, transpose (layers 1-2) into the resident h^T, and run the
      NEXT layer's phase A for this block immediately (overlaps B).
  - Final layer: single head, fp16 rows of 128 cols, log_softmax per block.

Pads: gather idx 0 (finite garbage), dstcode -1 => one-hot column all zero,
so pads contribute nothing to numerator or denominator.
"""

import os
import numpy as np

N = 10000
E = 320000
IN = 128
HID = 32
HEADS = 8
HC = HEADS * HID          # 256
OUT = 40
NEG = 0.2

NPAD = 10240              # 80 blocks of 128
NBLK_TOT = NPAD // 128    # 80
NCORES = 8
NB = NBLK_TOT // NCORES   # 10 blocks per core

ROW12 = 384               # xe row fp16 cols, layers 1-2: [xp 256 | al_s 8 | pad]
ROW3 = 128                # layer 3: [xp 40 | al_s 1 | pad]

NQ = int(os.environ.get("GAT_QUEUES", "3"))      # SWDGE queues for gathers
GBUFS = int(os.environ.get("GAT_GBUFS", "3"))    # gather tile double-buffering


# ----------------------------------------------------------------------------
# host-side preprocessing
# ----------------------------------------------------------------------------

def build_w_ext(W, a_src, a_dst, row):
    """W_ext[in, cols]: [W | W@As | W@Ad] (block-diag attention vectors)."""
    inn, hc = W.shape
    H, C = a_src.shape
    As = np.zeros((hc, H), np.float32)
    Ad = np.zeros((hc, H), np.float32)
    for h in range(H):
        As[h * C:(h + 1) * C, h] = a_src[h]
        Ad[h * C:(h + 1) * C, h] = a_dst[h]
    We = np.zeros((inn, row), np.float32)
    We[:, 0:hc] = W
    We[:, hc:hc + H] = W @ As
    We[:, hc + H:hc + 2 * H] = W @ Ad
    return We.astype(np.float16)


def preprocess(edge_index):
    """Chunk tables shared by all layers. Returns (CK, per-core arrays)."""
    src = np.concatenate([edge_index[0], np.arange(N, dtype=edge_index.dtype)])
    dst = np.concatenate([edge_index[1], np.arange(N, dtype=edge_index.dtype)])
    src = src.astype(np.int64)
    dst = dst.astype(np.int64)
    order = np.argsort(dst, kind="stable")
    ssrc, sdst = src[order], dst[order]
    blk = sdst // 128
    cnt = np.bincount(blk, minlength=NBLK_TOT)
    CK = int(np.ceil(cnt.max() / 128))
    S = CK * 128
    starts = np.concatenate([[0], np.cumsum(cnt)])

    gsrc = np.zeros((NBLK_TOT, S), np.int64)           # gather idx (by src)
    dstc = np.full((NBLK_TOT, S), -1.0, np.float32)    # dst - 128*block
    for b in range(NBLK_TOT):
        lo, hi = starts[b], starts[b + 1]
        n = hi - lo
        gsrc[b, :n] = ssrc[lo:hi]
        dstc[b, :n] = (sdst[lo:hi] - 128 * b).astype(np.float32)

    def wrap16(idx_flat):
        # [S] -> [128, S//16] int16; idx i -> [i%16, i//16], replicated to
        # all 8 GPSIMD partition groups.
        t16 = idx_flat.reshape(S // 16, 16).T.astype(np.int16)
        return np.tile(t16, (8, 1))

    cores = []
    for k in range(NCORES):
        bs = range(k * NB, (k + 1) * NB)
        gsrc_t = np.concatenate([wrap16(gsrc[b]) for b in bs], axis=1)
        # dstc_t[p, b*CK+c] = code of slot c*128+p in block b
        dc = dstc[k * NB:(k + 1) * NB].reshape(NB, CK, 128)
        dct = dc.transpose(2, 0, 1).reshape(128, NB * CK)
        # dstcB[q, ((b*CK)+c)*128 + p] = code of slot (p, c) -- replicated
        # along partitions q (for the transposed one-hot build)
        dcb = np.tile(dc.reshape(1, NB * CK * 128), (128, 1)).astype(np.float16)
        cores.append(dict(gsrc=gsrc_t,
                          dstc=np.ascontiguousarray(dct),
                          dstcB=np.ascontiguousarray(dcb)))
    return CK, cores


# ----------------------------------------------------------------------------
# bass program
# ----------------------------------------------------------------------------

def build_nc(CK):
    import concourse.bacc as bacc
    import concourse.mybir as mybir
    import concourse.tile as tile
    from concourse.library_config import mlp

    f32 = mybir.dt.float32
    f16 = mybir.dt.float16
    i16 = mybir.dt.int16
    Alu = mybir.AluOpType
    Act = mybir.ActivationFunctionType

    S = CK * 128

    nc = bacc.Bacc("TRN2", debug=False, num_swdge_queues=NQ)

    # inputs (per core)
    xT16 = nc.dram_tensor("xT16", [IN, NB * 128], f16, kind="ExternalInput")
    W1e = nc.dram_tensor("W1e", [IN, 272], f16, kind="ExternalInput")
    W2e = nc.dram_tensor("W2e", [HC, 272], f16, kind="ExternalInput")
    W3e = nc.dram_tensor("W3e", [HC, 64], f16, kind="ExternalInput")
    gsrc = nc.dram_tensor("gsrc", [128, NB * S // 16], i16, kind="ExternalInput")
    dstc = nc.dram_tensor("dstc", [128, NB * CK], f32, kind="ExternalInput")
    dstcB = nc.dram_tensor("dstcB", [128, NB * CK * 128], f16, kind="ExternalInput")
    iotaQ = nc.dram_tensor("iotaQ", [128, CK * 128], f16, kind="ExternalInput")
    iotaP = nc.dram_tensor("iotaP", [128, 8], f16, kind="ExternalInput")
    ident = nc.dram_tensor("ident", [128, 128], f16, kind="ExternalInput")
    b1r = nc.dram_tensor("b1r", [128, HC], f32, kind="ExternalInput")
    b2r = nc.dram_tensor("b2r", [128, HC], f32, kind="ExternalInput")
    b3r = nc.dram_tensor("b3r", [128, OUT], f32, kind="ExternalInput")

    out = nc.dram_tensor("out", [NB * 128, OUT], f32, kind="ExternalOutput")

    # scratch DRAM
    xe1s = nc.dram_tensor("xe1s", [NB * 128, ROW12], f16)
    xe1f = nc.dram_tensor("xe1f", [NPAD, ROW12], f16, addr_space="Shared")
    xe2s = nc.dram_tensor("xe2s", [NB * 128, ROW12], f16)
    xe2f = nc.dram_tensor("xe2f", [NPAD, ROW12], f16, addr_space="Shared")
    xe3s = nc.dram_tensor("xe3s", [NB * 128, ROW3], f16)
    xe3f = nc.dram_tensor("xe3f", [NPAD, ROW3], f16, addr_space="Shared")

    with tile.TileContext(nc) as tc:
        nc.gpsimd.load_library(mlp)
        with tc.tile_pool(name="const", bufs=1) as cpool, \
             tc.tile_pool(name="res", bufs=1) as rpool, \
             tc.tile_pool(name="dcb", bufs=2) as dcbpool, \
             tc.tile_pool(name="ohd", bufs=2) as ohpool, \
             tc.tile_pool(name="ohdt", bufs=2) as ohtpool, \
             tc.tile_pool(name="g", bufs=GBUFS) as gpool, \
             tc.tile_pool(name="small", bufs=2) as spool, \
             tc.tile_pool(name="post", bufs=2) as ppool, \
             tc.tile_pool(name="psA", bufs=2, space="PSUM") as psA, \
             tc.tile_pool(name="psB", bufs=2, space="PSUM") as psB, \
             tc.tile_pool(name="psT", bufs=2, space="PSUM") as psT, \
             tc.tile_pool(name="psAD", bufs=2, space="PSUM") as psAD:

            # constants resident in SBUF
            gsrc_t = cpool.tile([128, NB * S // 16], i16, tag="gsrc")
            nc.sync.dma_start(gsrc_t[:], gsrc[:])
            dstc_t = cpool.tile([128, NB * CK], f32, tag="dstc")
            nc.sync.dma_start(dstc_t[:], dstc[:])
            iotaQ_t = cpool.tile([128, CK * 128], f16, tag="iotaQ")
            nc.sync.dma_start(iotaQ_t[:], iotaQ[:])
            iotaP_t = cpool.tile([128, 8], f16, tag="iotaP")
            nc.sync.dma_start(iotaP_t[:], iotaP[:])
            ident_t = cpool.tile([128, 128], f16, tag="ident")
            nc.sync.dma_start(ident_t[:], ident[:])
            xT_t = cpool.tile([128, NB * 128], f16, tag="xT")
            nc.sync.dma_start(xT_t[:], xT16[:])
            w1_t = cpool.tile([128, 272], f16, tag="w1")
            nc.sync.dma_start(w1_t[:], W1e[:])
            w2_t = cpool.tile([128, 2, 272], f16, tag="w2")
            for kk in range(2):
                nc.sync.dma_start(w2_t[:, kk, :], W2e[kk * 128:(kk + 1) * 128, :])
            w3_t = cpool.tile([128, 2, 64], f16, tag="w3")
            for kk in range(2):
                nc.sync.dma_start(w3_t[:, kk, :], W3e[kk * 128:(kk + 1) * 128, :])
            b1_t = cpool.tile([128, HC], f32, tag="b1")
            nc.sync.dma_start(b1_t[:], b1r[:])
            b2_t = cpool.tile([128, HC], f32, tag="b2")
            nc.sync.dma_start(b2_t[:], b2r[:])
            b3_t = cpool.tile([128, OUT], f32, tag="b3")
            nc.sync.dma_start(b3_t[:], b3r[:])

            # resident tiles
            hT_t = rpool.tile([128, 2, NB, 128], f16, tag="hT")
            ald1_t = rpool.tile([128, NB, HEADS], f16, tag="ald1")
            ald2_t = rpool.tile([128, NB, HEADS], f16, tag="ald2")
            ald3_t = rpool.tile([128, NB, 1], f16, tag="ald3")

            def phase_a1():
                """xe1 shard = x_blk @ W1e for the core's own blocks."""
                for t in range(NB):
                    ps = psA.tile([128, 272], f32, tag="psA")
                    nc.tensor.matmul(ps[:], xT_t[:, t * 128:(t + 1) * 128],
                                     w1_t[:], start=True, stop=True)
                    xe_sb = ppool.tile([128, 264], f16, tag="xeA")
                    nc.vector.tensor_copy(xe_sb[:], ps[:, 0:264])
                    nc.vector.tensor_copy(ald1_t[:, t, :], ps[:, 264:272])
                    nc.sync.dma_start(xe1s[t * 128:(t + 1) * 128, 0:264], xe_sb[:])

            def phase_b(L, xe_f, ald_t, b_t, wn_t, ald_next, xe_next_s):
                """aggregation over this core's NB blocks (+ embedded next A)."""
                nh = HEADS if L < 3 else 1
                fe = HC if L < 3 else OUT
                row = ROW12 if L < 3 else ROW3
                for b in range(NB):
                    ofs = b * CK
                    # one-hot builds (independent of xe_f -> overlap AllGather)
                    dcb = dcbpool.tile([128, CK * 128], f16, tag="dcb")
                    nc.sync.dma_start(
                        dcb[:], dstcB[:, ofs * 128:(ofs + CK) * 128])
                    ohd = ohpool.tile([128, CK, 128], f16, tag="ohd")
                    nc.vector.tensor_tensor(
                        ohd[:],
                        dstc_t[:, ofs:ofs + CK].to_broadcast([128, CK, 128]),
                        iotaQ_t[:].rearrange("p (c q) -> p c q", q=128),
                        Alu.is_equal)
                    ohdt = ohtpool.tile([128, CK, 128], f16, tag="ohdt")
                    nc.vector.tensor_tensor(
                        ohdt[:],
                        dcb[:].rearrange("p (c q) -> p c q", q=128),
                        iotaP_t[:, 0:1].to_broadcast([128, CK, 128]),
                        Alu.is_equal)
                    # al_d per edge slot = ohT^T @ al_d_blk  (TensorE)
                    adps = psAD.tile([128, CK, HEADS], f32, tag="adps")
                    for c in range(CK):
                        nc.tensor.matmul(adps[:, c, 0:nh], ohdt[:, c, :],
                                         ald_t[:, b, 0:nh],
                                         start=True, stop=True)
                    # THE gather: [xp | al_s] rows by src
                    g = gpool.tile([128, CK, row], f16,
                                   tag="g" if L < 3 else "g3")
                    isl = gsrc_t[:, b * S // 16:(b + 1) * S // 16]
                    nc.gpsimd.dma_gather(
                        g[:], xe_f[:, 0:row], isl, S, S, row,
                        elem_step=row, single_packet=False,
                        queue_num=b % NQ)
                    # ee = exp(lrelu(al_s + al_d))
                    z = spool.tile([128, CK, nh], f32, tag="z")
                    nc.vector.tensor_tensor(
                        z[:], g[:, :, fe:fe + nh], adps[:, :, 0:nh], Alu.add)
                    zf = z[:].rearrange("p c h -> p (c h)")
                    nc.vector.scalar_tensor_tensor(
                        zf, zf, NEG, zf, Alu.mult, Alu.max)
                    ee = spool.tile([128, CK, nh], f16, tag="ee")
                    nc.scalar.activation(
                        ee[:].rearrange("p c h -> p (c h)"), zf, Act.Exp)
                    # scale features, stash ee next to them
                    if L < 3:
                        nc.vector.tensor_tensor(
                            g[:, :, 0:fe].rearrange("p c (h w) -> p c h w", w=HID),
                            g[:, :, 0:fe].rearrange("p c (h w) -> p c h w", w=HID),
                            ee[:].to_broadcast([128, CK, nh, HID]),
                            Alu.mult)
                    else:
                        nc.vector.tensor_tensor(
                            g[:, :, 0:fe], g[:, :, 0:fe],
                            ee[:].rearrange("p c h -> p (c h)").to_broadcast(
                                [128, CK, fe]),
                            Alu.mult)
                    nc.vector.tensor_copy(g[:, :, fe:fe + nh], ee[:])
                    # segment-sum via one-hot matmul
                    ps = psB.tile([128, 264], f32, tag="agg")
                    for c in range(CK):
                        nc.tensor.matmul(
                            ps[:, 0:fe + nh], ohd[:, c, :], g[:, c, 0:fe + nh],
                            start=(c == 0), stop=(c == CK - 1))
                    # divide by ee-sum, bias
                    r = spool.tile([128, nh], f32, tag="r")
                    nc.vector.tensor_scalar(
                        r[:], ps[:, fe:fe + nh], 1e-16, None, Alu.add)
                    nc.vector.reciprocal(r[:], r[:])
                    h = ppool.tile([128, fe], f32, tag="h")
                    if L < 3:
                        nc.vector.tensor_tensor(
                            h[:].rearrange("p (x w) -> p x w", w=HID),
                            ps[:, 0:fe].rearrange("p (x w) -> p x w", w=HID),
                            r[:].to_broadcast([128, nh, HID]),
                            Alu.mult)
                        nc.vector.tensor_tensor(h[:], h[:], b_t[:], Alu.add)
                        # ELU: relu(z) + exp(min(z,0)) - 1
                        t2 = ppool.tile([128, fe], f32, tag="elu")
                        nc.vector.tensor_scalar(t2[:], h[:], 0.0, None, Alu.min)
                        nc.scalar.activation(t2[:], t2[:], Act.Exp)
                        nc.vector.scalar_tensor_tensor(
                            h[:], h[:], 0.0, t2[:], Alu.max, Alu.add)
                        nc.vector.tensor_scalar(h[:], h[:], -1.0, None, Alu.add)
                        h16 = ppool.tile([128, fe], f16, tag="h16")
                        nc.vector.tensor_copy(h16[:], h[:])
                        for half in range(2):
                            pt = psT.tile([128, 128], f16, tag="tr")
                            nc.tensor.transpose(
                                pt[:], h16[:, half * 128:(half + 1) * 128],
                                ident_t[:])
                            nc.vector.tensor_copy(hT_t[:, half, b, :], pt[:])
                        # embedded next-layer phase A for this block
                        ncols = 272 if L == 1 else 64
                        wr = 264 if L == 1 else 41
                        psa = psA.tile([128, 272], f32, tag="psA")
                        for kk in range(2):
                            nc.tensor.matmul(
                                psa[:, 0:ncols], hT_t[:, kk, b, :],
                                wn_t[:, kk, 0:ncols],
                                start=(kk == 0), stop=(kk == 1))
                        xa = ppool.tile([128, 264], f16, tag="xeA")
                        nc.vector.tensor_copy(xa[:, 0:wr], psa[:, 0:wr])
                        nc.vector.tensor_copy(
                            ald_next[:, b, 0:(nh if L == 1 else 1)],
                            psa[:, wr:wr + (8 if L == 1 else 1)])
                        nc.sync.dma_start(
                            xe_next_s[b * 128:(b + 1) * 128, 0:wr],
                            xa[:, 0:wr])
                    else:
                        # log_softmax over the 40 classes
                        nc.vector.tensor_tensor(
                            h[:], ps[:, 0:fe],
                            r[:].to_broadcast([128, fe]), Alu.mult)
                        nc.vector.tensor_tensor(h[:], h[:], b_t[:], Alu.add)
                        m_t = spool.tile([128, 1], f32, tag="m")
                        nc.vector.tensor_reduce(
                            m_t[:], h[:], mybir.AxisListType.X, Alu.max)
                        nc.vector.tensor_tensor(
                            h[:], h[:], m_t[:].to_broadcast([128, fe]),
                            Alu.subtract)
                        x_t = ppool.tile([128, fe], f32, tag="exps")
                        s_t = spool.tile([128, 1], f32, tag="s")
                        nc.scalar.activation(
                            x_t[:], h[:], Act.Exp, accum_out=s_t[:])
                        l_t = spool.tile([128, 1], f32, tag="l")
                        nc.scalar.activation(l_t[:], s_t[:], Act.Ln)
                        nc.vector.tensor_tensor(
                            h[:], h[:], l_t[:].to_broadcast([128, fe]),
                            Alu.subtract)
                        nc.sync.dma_start(out[b * 128:(b + 1) * 128, :], h[:])

            def allgather(src_dram, dst_dram):
                nc.gpsimd.collective_compute(
                    "AllGather", mybir.AluOpType.bypass,
                    replica_groups=[list(range(NCORES))],
                    ins=[src_dram.ap().opt()], outs=[dst_dram.ap().opt()])

            phase_a1()
            allgather(xe1s, xe1f)
            phase_b(1, xe1f, ald1_t, b1_t, w2_t, ald2_t, xe2s)
            allgather(xe2s, xe2f)
            phase_b(2, xe2f, ald2_t, b2_t, w3_t, ald3_t, xe3s)
            allgather(xe3s, xe3f)
            phase_b(3, xe3f, ald3_t, b3_t, None, None, None)

    nc.compile()
    return nc


# ----------------------------------------------------------------------------
# entry point
# ----------------------------------------------------------------------------

LAST_EXEC_NS = None


def kernel(**inputs):
    from concourse.bass_utils import run_bass_kernel_spmd
    global LAST_EXEC_NS

    x = np.asarray(inputs["x"], np.float32)
    ei = np.asarray(inputs["edge_index"])
    CK, cores = preprocess(ei)

    xT16 = np.zeros((IN, NPAD), np.float16)
    xT16[:, 0:N] = x.T.astype(np.float16)
    W1en = build_w_ext(np.asarray(inputs["W1"], np.float32),
                       np.asarray(inputs["a_src1"], np.float32),
                       np.asarray(inputs["a_dst1"], np.float32), 272)
    W2en = build_w_ext(np.asarray(inputs["W2"], np.float32),
                       np.asarray(inputs["a_src2"], np.float32),
                       np.asarray(inputs["a_dst2"], np.float32), 272)
    W3en = np.zeros((HC, 64), np.float32)
    W3en[:, 0:OUT] = np.asarray(inputs["W3"], np.float32)
    W3en[:, OUT:OUT + 1] = (np.asarray(inputs["W3"], np.float32)
                            @ np.asarray(inputs["a_src3"], np.float32).reshape(OUT, 1))
    W3en[:, OUT + 1:OUT + 2] = (np.asarray(inputs["W3"], np.float32)
                                @ np.asarray(inputs["a_dst3"], np.float32).reshape(OUT, 1))
    W3en = W3en.astype(np.float16)

    iotaQ_n = np.tile(np.arange(128, dtype=np.float16), (128, CK))
    iotaP_n = np.tile(np.arange(128, dtype=np.float16).reshape(128, 1), (1, 8))
    ident_n = np.eye(128, dtype=np.float16)
    b1n = np.tile(np.asarray(inputs["b1"], np.float32), (128, 1))
    b2n = np.tile(np.asarray(inputs["b2"], np.float32), (128, 1))
    b3n = np.tile(np.asarray(inputs["b3"], np.float32), (128, 1))

    nc = build_nc(CK)
    in_maps = []
    for k in range(NCORES):
        in_maps.append({
            "xT16": np.ascontiguousarray(xT16[:, k * NB * 128:(k + 1) * NB * 128]),
            "W1e": W1en, "W2e": W2en, "W3e": W3en,
            "gsrc": cores[k]["gsrc"], "dstc": cores[k]["dstc"],
            "dstcB": cores[k]["dstcB"],
            "iotaQ": iotaQ_n, "iotaP": iotaP_n, "ident": ident_n,
            "b1r": b1n, "b2r": b2n, "b3r": b3n,
        })
    trace = bool(int(os.environ.get("GAT_TRACE", "0")))
    res = run_bass_kernel_spmd(nc, in_maps, list(range(NCORES)), trace=trace)
    LAST_EXEC_NS = res.exec_time_ns
    full = np.concatenate([res.results[k]["out"] for k in range(NCORES)], axis=0)
    return full[0:N].astype(np.float32)


# revision 8
# speedup vs baseline: 3.3372x; 1.3878x over previous
"""Trainium2 Bass kernel for 3-layer GAT (nn_MultiLayerGAT), v3.

Strategy (dst-node sharding, 8 cores, fp16 data path):
  - Add self-loops, sort edges by dst. Nodes padded to 10240 = 80 blocks of
    128; core k owns blocks [10k, 10k+10). Each block's edges padded to CK
    chunks of 128 slots (same CK for all cores => one SPMD program).
  - Phase A is SHARDED: each core computes xp_ext = h @ W_ext for its own 10
    blocks only, writing two [640, row] fp16 half-shards; two AllGathers
    (issued as soon as each half is complete, overlapping phase B of the
    previous layer) replicate the full xe to every core.  xe row =
    [xp | al_s | pad] fp16; al_d stays in a per-core resident SBUF tile.
    xe_full row layout is region-major: node (c, b, p) lives at row
    c*640 + b*128 + p for blocks b<5, and 5120 + c*640 + (b-5)*128 + p for
    b>=5 (so each AllGather writes one contiguous region); gather indices
    are host-remapped accordingly.
  - Phase B per dst block:
      The TRANSPOSED dst one-hot (ohdT[q, slot] = dstcode[slot]==q) is
      host-precomputed and DMA-streamed; the aggregation one-hot ohd is
      derived on-chip by PE transposes (batched PSUM->SBUF copies).
      al_d per edge slot = ohdT^T @ al_d_blk on TensorE (no dma_gather).
      ONE gpsimd dma_gather per block fetches [xp|al_s] rows by src (768B
      fp16 elements).  Gathers rotate across 4 SWDGE queues so descriptor
      generation runs concurrently on different Q7 core pairs (desc-gen at
      ~41us/4352 idxs is otherwise the serial critical path; measured 2.5x
      faster with 4 queues).
      ee = exp(lrelu(al_s + al_d)); scale gathered xp by ee; segment-sum
      AND the ee denominator via interleaved one-hot matmuls on TensorE
      (fp16, fp32 PSUM): ps[:,0:fe] += ohd_c^T @ (ee*G)_c and
      ps[:,fe:fe+nh] += ohd_c^T @ ee_c.
      h = ps*recip(s+eps) + bias; layers 1-2 store h' = elu(h)+1 = max(h,0)
      + exp(min(h,0)) (the -1 is folded into the next layer's phase A as a
      host-precomputed column-sum correction row), transpose into the
      resident h^T, and run the next layer's phase A for this block
      immediately.
  - Final layer: single head, fp16 rows of 128 cols, log_softmax per block.

Pads: gather idx 0 (finite), dstcode -1 => one-hot column all zero, so pads
contribute nothing to numerator or denominator.
"""

import os
import numpy as np

N = 10000
E = 320000
IN = 128
HID = 32
HEADS = 8
HC = HEADS * HID          # 256
OUT = 40
NEG = 0.2

NPAD = 10240              # 80 blocks of 128
NBLK_TOT = NPAD // 128    # 80
NCORES = 8
NB = NBLK_TOT // NCORES   # 10 blocks per core
NBH = NB // 2             # blocks per half-shard

ROW12 = 384               # xe row fp16 cols, layers 1-2: [xp 256 | al_s 8 | pad]
ROW3 = 128                # layer 3: [xp 40 | al_s 1 | pad]

NQ = int(os.environ.get("GAT_QUEUES", "4"))      # SWDGE queues for gathers
GBUFS = int(os.environ.get("GAT_GBUFS", "4"))    # gather tiles in flight


def rowmap(node):
    """xe_full row of a global (padded) node id, region-major halves."""
    c, r = np.divmod(node, NB * 128)
    b, p = np.divmod(r, 128)
    lo = b < NBH
    return np.where(lo,
                    c * (NBH * 128) + b * 128 + p,
                    NCORES * NBH * 128 + c * (NBH * 128) + (b - NBH) * 128 + p)


# ----------------------------------------------------------------------------
# host-side preprocessing
# ----------------------------------------------------------------------------

def build_w_ext(W, a_src, a_dst, row):
    """W_ext[in, cols]: [W | W@As | W@Ad] (block-diag attention vectors)."""
    inn, hc = W.shape
    H, C = a_src.shape
    As = np.zeros((hc, H), np.float32)
    Ad = np.zeros((hc, H), np.float32)
    for h in range(H):
        As[h * C:(h + 1) * C, h] = a_src[h]
        Ad[h * C:(h + 1) * C, h] = a_dst[h]
    We = np.zeros((inn, row), np.float32)
    We[:, 0:hc] = W
    We[:, hc:hc + H] = W @ As
    We[:, hc + H:hc + 2 * H] = W @ Ad
    return We.astype(np.float16)


def preprocess(edge_index):
    """Chunk tables shared by all layers. Returns (CK, per-core arrays)."""
    src = np.concatenate([edge_index[0], np.arange(N, dtype=edge_index.dtype)])
    dst = np.concatenate([edge_index[1], np.arange(N, dtype=edge_index.dtype)])
    src = src.astype(np.int64)
    dst = dst.astype(np.int64)
    order = np.argsort(dst, kind="stable")
    ssrc, sdst = src[order], dst[order]
    blk = sdst // 128
    cnt = np.bincount(blk, minlength=NBLK_TOT)
    CK = int(np.ceil(cnt.max() / 128))
    S = CK * 128
    starts = np.concatenate([[0], np.cumsum(cnt)])

    gsrc = np.zeros((NBLK_TOT, S), np.int64)           # gather row (by src)
    dstc = np.full((NBLK_TOT, S), -1, np.int32)        # dst - 128*block
    srows = rowmap(ssrc)
    for b in range(NBLK_TOT):
        lo, hi = starts[b], starts[b + 1]
        n = hi - lo
        gsrc[b, :n] = srows[lo:hi]
        dstc[b, :n] = (sdst[lo:hi] - 128 * b).astype(np.int32)

    def wrap16(idx_flat):
        t16 = idx_flat.reshape(S // 16, 16).T.astype(np.int16)
        return np.tile(t16, (8, 1))

    qvec = np.arange(128, dtype=np.int32).reshape(128, 1)
    cores = []
    for k in range(NCORES):
        bs = range(k * NB, (k + 1) * NB)
        gsrc_t = np.concatenate([wrap16(gsrc[b]) for b in bs], axis=1)
        # ohdT[q, (b*CK + c)*128 + p] = (dstc[b, c*128+p] == q), fp16
        codes = dstc[k * NB:(k + 1) * NB].reshape(1, NB * S)
        ohdT = (qvec == codes).astype(np.float16)
        cores.append(dict(gsrc=gsrc_t, ohdT=np.ascontiguousarray(ohdT)))
    return CK, cores


# ----------------------------------------------------------------------------
# bass program
# ----------------------------------------------------------------------------

def build_nc(CK):
    import concourse.bacc as bacc
    import concourse.mybir as mybir
    import concourse.tile as tile
    from concourse.library_config import mlp

    f32 = mybir.dt.float32
    f16 = mybir.dt.float16
    i16 = mybir.dt.int16
    Alu = mybir.AluOpType
    Act = mybir.ActivationFunctionType

    S = CK * 128
    HR = NBH * 128            # rows per half-shard (640)

    nc = bacc.Bacc("TRN2", debug=False, num_swdge_queues=NQ)

    # inputs (per core)
    xT16 = nc.dram_tensor("xT16", [IN, NB * 128], f16, kind="ExternalInput")
    W1e = nc.dram_tensor("W1e", [IN, 272], f16, kind="ExternalInput")
    W2e = nc.dram_tensor("W2e", [HC, 272], f16, kind="ExternalInput")
    W3e = nc.dram_tensor("W3e", [HC, 64], f16, kind="ExternalInput")
    gsrc = nc.dram_tensor("gsrc", [128, NB * S // 16], i16, kind="ExternalInput")
    ohdT = nc.dram_tensor("ohdT", [128, NB * S], f16, kind="ExternalInput")
    ident = nc.dram_tensor("ident", [128, 128], f16, kind="ExternalInput")
    b1r = nc.dram_tensor("b1r", [128, HC], f32, kind="ExternalInput")
    b2r = nc.dram_tensor("b2r", [128, HC], f32, kind="ExternalInput")
    b3r = nc.dram_tensor("b3r", [128, OUT], f32, kind="ExternalInput")
    c2r = nc.dram_tensor("c2r", [128, 272], f32, kind="ExternalInput")
    c3r = nc.dram_tensor("c3r", [128, 64], f32, kind="ExternalInput")

    out = nc.dram_tensor("out", [NB * 128, OUT], f32, kind="ExternalOutput")

    # scratch DRAM: two half-shards + two-region full tensors per layer
    def xe_pair(name, row):
        sa = nc.dram_tensor(name + "sa", [HR, row], f16)
        sb = nc.dram_tensor(name + "sb", [HR, row], f16)
        ff = nc.dram_tensor(name + "f", [NPAD, row], f16, addr_space="Shared")
        return sa, sb, ff

    xe1sa, xe1sb, xe1f = xe_pair("xe1", ROW12)
    xe2sa, xe2sb, xe2f = xe_pair("xe2", ROW12)
    xe3sa, xe3sb, xe3f = xe_pair("xe3", ROW3)

    with tile.TileContext(nc) as tc:
        nc.gpsimd.load_library(mlp)
        with tc.tile_pool(name="const", bufs=1) as cpool, \
             tc.tile_pool(name="res", bufs=1) as rpool, \
             tc.tile_pool(name="oht", bufs=2) as ohtpool, \
             tc.tile_pool(name="ohd", bufs=2) as ohpool, \
             tc.tile_pool(name="g", bufs=GBUFS) as gpool, \
             tc.tile_pool(name="g3", bufs=3) as g3pool, \
             tc.tile_pool(name="small", bufs=2) as spool, \
             tc.tile_pool(name="post", bufs=2) as ppool, \
             tc.tile_pool(name="psA", bufs=2, space="PSUM") as psA, \
             tc.tile_pool(name="psB", bufs=2, space="PSUM") as psB, \
             tc.tile_pool(name="psT", bufs=2, space="PSUM") as psT, \
             tc.tile_pool(name="psAD", bufs=2, space="PSUM") as psAD:

            gsrc_t = cpool.tile([128, NB * S // 16], i16, tag="gsrc")
            nc.sync.dma_start(gsrc_t[:], gsrc[:])
            ident_t = cpool.tile([128, 128], f16, tag="ident")
            nc.sync.dma_start(ident_t[:], ident[:])
            xT_t = cpool.tile([128, NB * 128], f16, tag="xT")
            nc.sync.dma_start(xT_t[:], xT16[:])
            w1_t = cpool.tile([128, 272], f16, tag="w1")
            nc.sync.dma_start(w1_t[:], W1e[:])
            w2_t = cpool.tile([128, 2, 272], f16, tag="w2")
            for kk in range(2):
                nc.sync.dma_start(w2_t[:, kk, :], W2e[kk * 128:(kk + 1) * 128, :])
            w3_t = cpool.tile([128, 2, 64], f16, tag="w3")
            for kk in range(2):
                nc.sync.dma_start(w3_t[:, kk, :], W3e[kk * 128:(kk + 1) * 128, :])
            b1_t = cpool.tile([128, HC], f32, tag="b1")
            nc.sync.dma_start(b1_t[:], b1r[:])
            b2_t = cpool.tile([128, HC], f32, tag="b2")
            nc.sync.dma_start(b2_t[:], b2r[:])
            b3_t = cpool.tile([128, OUT], f32, tag="b3")
            nc.sync.dma_start(b3_t[:], b3r[:])
            c2_t = cpool.tile([128, 272], f32, tag="c2")
            nc.sync.dma_start(c2_t[:], c2r[:])
            c3_t = cpool.tile([128, 64], f32, tag="c3")
            nc.sync.dma_start(c3_t[:], c3r[:])

            hT_t = rpool.tile([128, 2, NB, 128], f16, tag="hT")
            ald1_t = rpool.tile([128, NB, HEADS], f16, tag="ald1")
            ald2_t = rpool.tile([128, NB, HEADS], f16, tag="ald2")
            ald3_t = rpool.tile([128, NB, 1], f16, tag="ald3")

            def shard_write(xe_sa, xe_sb, b, src_ap, wr):
                tgt = xe_sa if b < NBH else xe_sb
                r0 = (b % NBH) * 128
                nc.sync.dma_start(tgt[r0:r0 + 128, 0:wr], src_ap)

            def allgather(src_dram, dst_ap):
                nc.gpsimd.collective_compute(
                    "AllGather", mybir.AluOpType.bypass,
                    replica_groups=[list(range(NCORES))],
                    ins=[src_dram.ap().opt()], outs=[dst_ap.opt()])

            def phase_a1():
                for t in range(NB):
                    ps = psA.tile([128, 272], f32, tag="psA")
                    nc.tensor.matmul(ps[:], xT_t[:, t * 128:(t + 1) * 128],
                                     w1_t[:], start=True, stop=True)
                    xa = ppool.tile([128, 264], f16, tag="xeA")
                    nc.vector.tensor_copy(xa[:], ps[:, 0:264])
                    nc.vector.tensor_copy(ald1_t[:, t, :], ps[:, 264:272])
                    shard_write(xe1sa, xe1sb, t, xa[:], 264)
                    if t == NBH - 1:
                        allgather(xe1sa, xe1f[0:NCORES * HR, :])
                allgather(xe1sb, xe1f[NCORES * HR:NPAD, :])

            def phase_b(L, xe_f, ald_t, b_t, wn_t, corr_t, ald_next,
                        xe_nsa, xe_nsb):
                nh = HEADS if L < 3 else 1
                fe = HC if L < 3 else OUT
                row = ROW12 if L < 3 else ROW3
                for b in range(NB):
                    ofs = b * S
                    # transposed one-hot from host; derive ohd by PE transpose
                    oht = ohtpool.tile([128, CK, 128], f16, tag="oht")
                    nc.sync.dma_start(
                        oht[:].rearrange("p c q -> p (c q)"),
                        ohdT[:, ofs:ofs + S])
                    ohd = ohpool.tile([128, CK, 128], f16, tag="ohd")
                    for grp in range((CK + 3) // 4):
                        c0 = grp * 4
                        cw = min(4, CK - c0)
                        pt = psT.tile([128, 4, 128], f16, tag="tr4")
                        for j in range(cw):
                            nc.tensor.transpose(
                                pt[:, j, :], oht[:, c0 + j, :], ident_t[:])
                        nc.vector.tensor_copy(
                            ohd[:, c0:c0 + cw, :], pt[:, 0:cw, :])
                    # al_d per edge slot = ohdT^T @ al_d_blk  (TensorE)
                    adps = psAD.tile([128, CK, HEADS], f32, tag="adps")
                    for c in range(CK):
                        nc.tensor.matmul(adps[:, c, 0:nh], oht[:, c, :],
                                         ald_t[:, b, 0:nh],
                                         start=True, stop=True)
                    # THE gather: [xp | al_s] rows by src
                    g = (gpool if L < 3 else g3pool).tile(
                        [128, CK, row], f16, tag="g" if L < 3 else "g3")
                    isl = gsrc_t[:, b * S // 16:(b + 1) * S // 16]
                    nc.gpsimd.dma_gather(
                        g[:], xe_f[:, 0:row], isl, S, S, row,
                        elem_step=row, single_packet=False,
                        queue_num=b % NQ)
                    # ee = exp(lrelu(al_s + al_d))
                    z = spool.tile([128, CK, nh], f32, tag="z")
                    nc.vector.tensor_tensor(
                        z[:], g[:, :, fe:fe + nh], adps[:, :, 0:nh], Alu.add)
                    zf = z[:].rearrange("p c h -> p (c h)")
                    nc.vector.scalar_tensor_tensor(
                        zf, zf, NEG, zf, Alu.mult, Alu.max)
                    ee = spool.tile([128, CK, nh], f16, tag="ee")
                    nc.scalar.activation(
                        ee[:].rearrange("p c h -> p (c h)"), zf, Act.Exp)
                    # scale features by ee
                    if L < 3:
                        nc.vector.tensor_tensor(
                            g[:, :, 0:fe].rearrange("p c (h w) -> p c h w", w=HID),
                            g[:, :, 0:fe].rearrange("p c (h w) -> p c h w", w=HID),
                            ee[:].to_broadcast([128, CK, nh, HID]),
                            Alu.mult)
                    else:
                        nc.vector.tensor_tensor(
                            g[:, :, 0:fe], g[:, :, 0:fe],
                            ee[:].rearrange("p c h -> p (c h)").to_broadcast(
                                [128, CK, fe]),
                            Alu.mult)
                    # segment-sum + ee denominator via interleaved matmuls
                    ps = psB.tile([128, 264], f32, tag="agg")
                    for c in range(CK):
                        nc.tensor.matmul(
                            ps[:, 0:fe], ohd[:, c, :], g[:, c, 0:fe],
                            start=(c == 0), stop=(c == CK - 1))
                    for c in range(CK):
                        nc.tensor.matmul(
                            ps[:, fe:fe + nh], ohd[:, c, :], ee[:, c, :],
                            start=(c == 0), stop=(c == CK - 1))
                    # h = ps * 1/(s+eps) + bias
                    r = spool.tile([128, nh], f32, tag="r")
                    nc.vector.reciprocal(r[:], ps[:, fe:fe + nh])
                    h = ppool.tile([128, fe], f32, tag="h")
                    if L < 3:
                        nc.vector.tensor_tensor(
                            h[:].rearrange("p (x w) -> p x w", w=HID),
                            ps[:, 0:fe].rearrange("p (x w) -> p x w", w=HID),
                            r[:].to_broadcast([128, nh, HID]),
                            Alu.mult)
                        nc.vector.tensor_tensor(h[:], h[:], b_t[:], Alu.add)
                        # h' = elu(h)+1 = max(h,0) + exp(min(h,0))
                        t2 = ppool.tile([128, fe], f32, tag="elu")
                        nc.vector.scalar_tensor_tensor(
                            t2[:], h[:], 0.0, h[:], Alu.min, Alu.min)
                        nc.scalar.activation(t2[:], t2[:], Act.Exp)
                        h16 = ppool.tile([128, fe], f16, tag="h16")
                        nc.vector.scalar_tensor_tensor(
                            h16[:], h[:], 0.0, t2[:], Alu.max, Alu.add)
                        for half in range(2):
                            pt = psT.tile([128, 4, 128], f16, tag="tr4")
                            nc.tensor.transpose(
                                pt[:, 0, :], h16[:, half * 128:(half + 1) * 128],
                                ident_t[:])
                            nc.vector.tensor_copy(hT_t[:, half, b, :],
                                                  pt[:, 0, :])
                        # embedded next-layer phase A (h' @ W - colsum(W))
                        ncols = 272 if L == 1 else 64
                        wr = 264 if L == 1 else 41
                        nhn = 8 if L == 1 else 1
                        psa = psA.tile([128, 272], f32, tag="psA")
                        for kk in range(2):
                            nc.tensor.matmul(
                                psa[:, 0:ncols], hT_t[:, kk, b, :],
                                wn_t[:, kk, 0:ncols],
                                start=(kk == 0), stop=(kk == 1))
                        xa = ppool.tile([128, 264], f16, tag="xeA")
                        nc.vector.tensor_tensor(
                            xa[:, 0:wr], psa[:, 0:wr], corr_t[:, 0:wr],
                            Alu.subtract)
                        nc.vector.tensor_tensor(
                            ald_next[:, b, 0:nhn], psa[:, wr:wr + nhn],
                            corr_t[:, wr:wr + nhn], Alu.subtract)
                        shard_write(xe_nsa, xe_nsb, b, xa[:, 0:wr], wr)
                        if b == NBH - 1:
                            allgather(xe_nsa,
                                      (xe2f if L == 1 else xe3f)[0:NCORES * HR, :])
                        elif b == NB - 1:
                            allgather(xe_nsb,
                                      (xe2f if L == 1 else xe3f)[NCORES * HR:NPAD, :])
                    else:
                        # log_softmax over the 40 classes
                        nc.vector.tensor_tensor(
                            h[:], ps[:, 0:fe],
                            r[:].to_broadcast([128, fe]), Alu.mult)
                        nc.vector.tensor_tensor(h[:], h[:], b_t[:], Alu.add)
                        m_t = spool.tile([128, 1], f32, tag="m")
                        nc.vector.tensor_reduce(
                            m_t[:], h[:], mybir.AxisListType.X, Alu.max)
                        nc.vector.tensor_tensor(
                            h[:], h[:], m_t[:].to_broadcast([128, fe]),
                            Alu.subtract)
                        x_t = ppool.tile([128, fe], f32, tag="exps")
                        s_t = spool.tile([128, 1], f32, tag="s")
                        nc.scalar.activation(
                            x_t[:], h[:], Act.Exp, accum_out=s_t[:])
                        l_t = spool.tile([128, 1], f32, tag="l")
                        nc.scalar.activation(l_t[:], s_t[:], Act.Ln)
                        nc.vector.tensor_tensor(
                            h[:], h[:], l_t[:].to_broadcast([128, fe]),
                            Alu.subtract)
                        nc.sync.dma_start(out[b * 128:(b + 1) * 128, :], h[:])

            phase_a1()
            phase_b(1, xe1f, ald1_t, b1_t, w2_t, c2_t, ald2_t, xe2sa, xe2sb)
            phase_b(2, xe2f, ald2_t, b2_t, w3_t, c3_t, ald3_t, xe3sa, xe3sb)
            phase_b(3, xe3f, ald3_t, b3_t, None, None, None, None, None)

    nc.compile()
    return nc


# ----------------------------------------------------------------------------
# entry point
# ----------------------------------------------------------------------------

LAST_EXEC_NS = None


def kernel(**inputs):
    from concourse.bass_utils import run_bass_kernel_spmd
    global LAST_EXEC_NS

    x = np.asarray(inputs["x"], np.float32)
    ei = np.asarray(inputs["edge_index"])
    CK, cores = preprocess(ei)

    xT16 = np.zeros((IN, NPAD), np.float16)
    xT16[:, 0:N] = x.T.astype(np.float16)
    W1en = build_w_ext(np.asarray(inputs["W1"], np.float32),
                       np.asarray(inputs["a_src1"], np.float32),
                       np.asarray(inputs["a_dst1"], np.float32), 272)
    W2en = build_w_ext(np.asarray(inputs["W2"], np.float32),
                       np.asarray(inputs["a_src2"], np.float32),
                       np.asarray(inputs["a_dst2"], np.float32), 272)
    W3_ = np.asarray(inputs["W3"], np.float32)
    W3en = np.zeros((HC, 64), np.float32)
    W3en[:, 0:OUT] = W3_
    W3en[:, OUT:OUT + 1] = W3_ @ np.asarray(inputs["a_src3"], np.float32).reshape(OUT, 1)
    W3en[:, OUT + 1:OUT + 2] = W3_ @ np.asarray(inputs["a_dst3"], np.float32).reshape(OUT, 1)
    W3en = W3en.astype(np.float16)

    # ELU -1 fold: colsum correction rows for the next layer's W_ext
    c2n = np.tile(W2en.astype(np.float32).sum(axis=0), (128, 1)).astype(np.float32)
    c3n = np.tile(W3en.astype(np.float32).sum(axis=0), (128, 1)).astype(np.float32)

    ident_n = np.eye(128, dtype=np.float16)
    b1n = np.tile(np.asarray(inputs["b1"], np.float32), (128, 1))
    b2n = np.tile(np.asarray(inputs["b2"], np.float32), (128, 1))
    b3n = np.tile(np.asarray(inputs["b3"], np.float32), (128, 1))

    nc = build_nc(CK)
    in_maps = []
    for k in range(NCORES):
        in_maps.append({
            "xT16": np.ascontiguousarray(xT16[:, k * NB * 128:(k + 1) * NB * 128]),
            "W1e": W1en, "W2e": W2en, "W3e": W3en,
            "gsrc": cores[k]["gsrc"], "ohdT": cores[k]["ohdT"],
            "ident": ident_n,
            "b1r": b1n, "b2r": b2n, "b3r": b3n, "c2r": c2n, "c3r": c3n,
        })
    trace = bool(int(os.environ.get("GAT_TRACE", "0")))
    res = run_bass_kernel_spmd(nc, in_maps, list(range(NCORES)), trace=trace)
    LAST_EXEC_NS = res.exec_time_ns
    full = np.concatenate([res.results[k]["out"] for k in range(NCORES)], axis=0)
    return full[0:N].astype(np.float32)


# revision 10
# speedup vs baseline: 3.6818x; 1.1033x over previous
"""Trainium2 Bass kernel for 3-layer GAT (nn_MultiLayerGAT), v3.

Strategy (dst-node sharding, 8 cores, fp16 data path):
  - Add self-loops, sort edges by dst. Nodes padded to 10240 = 80 blocks of
    128; core k owns blocks [10k, 10k+10). Each block's edges padded to CK
    chunks of 128 slots (same CK for all cores => one SPMD program).
  - Phase A is SHARDED: each core computes xp_ext = h @ W_ext for its own 10
    blocks only, writing two [640, row] fp16 half-shards; two AllGathers
    (issued as soon as each half is complete, overlapping phase B of the
    previous layer) replicate the full xe to every core.  xe row =
    [xp | al_s | pad] fp16; al_d stays in a per-core resident SBUF tile.
    xe_full row layout is region-major: node (c, b, p) lives at row
    c*640 + b*128 + p for blocks b<5, and 5120 + c*640 + (b-5)*128 + p for
    b>=5 (so each AllGather writes one contiguous region); gather indices
    are host-remapped accordingly.
  - Phase B per dst block:
      The TRANSPOSED dst one-hot (ohdT[q, slot] = dstcode[slot]==q) is
      host-precomputed and DMA-streamed; the aggregation one-hot ohd is
      derived on-chip by PE transposes (batched PSUM->SBUF copies).
      al_d per edge slot = ohdT^T @ al_d_blk on TensorE (no dma_gather).
      ONE gpsimd dma_gather per block fetches [xp|al_s] rows by src (768B
      fp16 elements).  Gathers rotate across 4 SWDGE queues so descriptor
      generation runs concurrently on different Q7 core pairs (desc-gen at
      ~41us/4352 idxs is otherwise the serial critical path; measured 2.5x
      faster with 4 queues).
      ee = exp(lrelu(al_s + al_d)); scale gathered xp by ee; segment-sum
      AND the ee denominator via interleaved one-hot matmuls on TensorE
      (fp16, fp32 PSUM): ps[:,0:fe] += ohd_c^T @ (ee*G)_c and
      ps[:,fe:fe+nh] += ohd_c^T @ ee_c.
      h = ps*recip(s+eps) + bias; layers 1-2 store h' = elu(h)+1 = max(h,0)
      + exp(min(h,0)) (the -1 is folded into the next layer's phase A as a
      host-precomputed column-sum correction row), transpose into the
      resident h^T, and run the next layer's phase A for this block
      immediately.
  - Final layer: single head, fp16 rows of 128 cols, log_softmax per block.

Pads: gather idx 0 (finite), dstcode -1 => one-hot column all zero, so pads
contribute nothing to numerator or denominator.
"""

import os
import numpy as np

N = 10000
E = 320000
IN = 128
HID = 32
HEADS = 8
HC = HEADS * HID          # 256
OUT = 40
NEG = 0.2

NPAD = 10240              # 80 blocks of 128
NBLK_TOT = NPAD // 128    # 80
NCORES = 8
NB = NBLK_TOT // NCORES   # 10 blocks per core
NBH = NB // 2             # blocks per half-shard

ROW12 = 384               # xe row fp16 cols, layers 1-2: [xp 256 | al_s 8 | pad]
ROW3 = 128                # layer 3: [xp 40 | al_s 1 | pad]

NQ = int(os.environ.get("GAT_QUEUES", "4"))      # SWDGE queues for gathers
GBUFS = int(os.environ.get("GAT_GBUFS", "4"))    # gather tiles in flight


def rowmap(node):
    """xe_full row of a global (padded) node id, region-major halves."""
    c, r = np.divmod(node, NB * 128)
    b, p = np.divmod(r, 128)
    lo = b < NBH
    return np.where(lo,
                    c * (NBH * 128) + b * 128 + p,
                    NCORES * NBH * 128 + c * (NBH * 128) + (b - NBH) * 128 + p)


# ----------------------------------------------------------------------------
# host-side preprocessing
# ----------------------------------------------------------------------------

def build_w_ext(W, a_src, a_dst, row):
    """W_ext[in, cols]: [W | W@As | W@Ad] (block-diag attention vectors)."""
    inn, hc = W.shape
    H, C = a_src.shape
    As = np.zeros((hc, H), np.float32)
    Ad = np.zeros((hc, H), np.float32)
    for h in range(H):
        As[h * C:(h + 1) * C, h] = a_src[h]
        Ad[h * C:(h + 1) * C, h] = a_dst[h]
    We = np.zeros((inn, row), np.float32)
    We[:, 0:hc] = W
    We[:, hc:hc + H] = W @ As
    We[:, hc + H:hc + 2 * H] = W @ Ad
    return We.astype(np.float16)


def preprocess(edge_index):
    """Chunk tables shared by all layers. Returns (CK, per-core arrays)."""
    src = np.concatenate([edge_index[0], np.arange(N, dtype=edge_index.dtype)])
    dst = np.concatenate([edge_index[1], np.arange(N, dtype=edge_index.dtype)])
    src = src.astype(np.int64)
    dst = dst.astype(np.int64)
    order = np.argsort(dst, kind="stable")
    ssrc, sdst = src[order], dst[order]
    blk = sdst // 128
    cnt = np.bincount(blk, minlength=NBLK_TOT)
    CK = int(np.ceil(cnt.max() / 128))
    S = CK * 128
    starts = np.concatenate([[0], np.cumsum(cnt)])

    gsrc = np.zeros((NBLK_TOT, S), np.int64)           # gather row (by src)
    dstc = np.full((NBLK_TOT, S), -1, np.int32)        # dst - 128*block
    srows = rowmap(ssrc)
    for b in range(NBLK_TOT):
        lo, hi = starts[b], starts[b + 1]
        n = hi - lo
        gsrc[b, :n] = srows[lo:hi]
        dstc[b, :n] = (sdst[lo:hi] - 128 * b).astype(np.int32)

    def wrap16(idx_flat):
        t16 = idx_flat.reshape(S // 16, 16).T.astype(np.int16)
        return np.tile(t16, (8, 1))

    qvec = np.arange(128, dtype=np.int32).reshape(128, 1)
    cores = []
    for k in range(NCORES):
        bs = range(k * NB, (k + 1) * NB)
        gsrc_t = np.concatenate([wrap16(gsrc[b]) for b in bs], axis=1)
        # ohdT[q, (b*CK + c)*128 + p] = (dstc[b, c*128+p] == q), fp16
        codes = dstc[k * NB:(k + 1) * NB].reshape(1, NB * S)
        ohdT = (qvec == codes).astype(np.float16)
        cores.append(dict(gsrc=gsrc_t, ohdT=np.ascontiguousarray(ohdT)))
    return CK, cores


# ----------------------------------------------------------------------------
# bass program
# ----------------------------------------------------------------------------

def build_nc(CK):
    import concourse.bacc as bacc
    import concourse.mybir as mybir
    import concourse.tile as tile
    from concourse.library_config import mlp

    f32 = mybir.dt.float32
    f16 = mybir.dt.float16
    i16 = mybir.dt.int16
    Alu = mybir.AluOpType
    Act = mybir.ActivationFunctionType

    S = CK * 128
    HR = NBH * 128            # rows per half-shard (640)

    nc = bacc.Bacc("TRN2", debug=False, num_swdge_queues=NQ)

    # inputs (per core)
    xT16 = nc.dram_tensor("xT16", [IN, NB * 128], f16, kind="ExternalInput")
    xTf = nc.dram_tensor("xTf", [IN, NPAD], f16, kind="ExternalInput")
    W1e = nc.dram_tensor("W1e", [IN, 272], f16, kind="ExternalInput")
    W2e = nc.dram_tensor("W2e", [HC, 272], f16, kind="ExternalInput")
    W3e = nc.dram_tensor("W3e", [HC, 64], f16, kind="ExternalInput")
    gsrc = nc.dram_tensor("gsrc", [128, NB * S // 16], i16, kind="ExternalInput")
    ohdT = nc.dram_tensor("ohdT", [128, NB * S], f16, kind="ExternalInput")
    ident = nc.dram_tensor("ident", [128, 128], f16, kind="ExternalInput")
    b1r = nc.dram_tensor("b1r", [128, HC], f32, kind="ExternalInput")
    b2r = nc.dram_tensor("b2r", [128, HC], f32, kind="ExternalInput")
    b3r = nc.dram_tensor("b3r", [128, OUT], f32, kind="ExternalInput")
    c2r = nc.dram_tensor("c2r", [128, 272], f32, kind="ExternalInput")
    c3r = nc.dram_tensor("c3r", [128, 64], f32, kind="ExternalInput")

    out = nc.dram_tensor("out", [NB * 128, OUT], f32, kind="ExternalOutput")

    # scratch DRAM: two half-shards + two-region full tensors per layer
    def xe_pair(name, row):
        sa = nc.dram_tensor(name + "sa", [HR, row], f16)
        sb = nc.dram_tensor(name + "sb", [HR, row], f16)
        ff = nc.dram_tensor(name + "f", [NPAD, row], f16, addr_space="Shared")
        return sa, sb, ff

    xe1f = nc.dram_tensor("xe1f", [NPAD, ROW12], f16)
    xe2sa, xe2sb, xe2f = xe_pair("xe2", ROW12)
    xe3sa, xe3sb, xe3f = xe_pair("xe3", ROW3)

    with tile.TileContext(nc) as tc:
        nc.gpsimd.load_library(mlp)
        with tc.tile_pool(name="const", bufs=1) as cpool, \
             tc.tile_pool(name="res", bufs=1) as rpool, \
             tc.tile_pool(name="oht", bufs=2) as ohtpool, \
             tc.tile_pool(name="ohd", bufs=2) as ohpool, \
             tc.tile_pool(name="g", bufs=GBUFS) as gpool, \
             tc.tile_pool(name="g3", bufs=3) as g3pool, \
             tc.tile_pool(name="small", bufs=2) as spool, \
             tc.tile_pool(name="post", bufs=2) as ppool, \
             tc.tile_pool(name="psA", bufs=2, space="PSUM") as psA, \
             tc.tile_pool(name="psB", bufs=2, space="PSUM") as psB, \
             tc.tile_pool(name="psT", bufs=2, space="PSUM") as psT, \
             tc.tile_pool(name="psAD", bufs=2, space="PSUM") as psAD:

            gsrc_t = cpool.tile([128, NB * S // 16], i16, tag="gsrc")
            nc.sync.dma_start(gsrc_t[:], gsrc[:])
            ident_t = cpool.tile([128, 128], f16, tag="ident")
            nc.sync.dma_start(ident_t[:], ident[:])
            xT_t = cpool.tile([128, NB * 128], f16, tag="xT")
            nc.sync.dma_start(xT_t[:], xT16[:])

            w1_t = cpool.tile([128, 272], f16, tag="w1")
            nc.sync.dma_start(w1_t[:], W1e[:])
            w2_t = cpool.tile([128, 2, 272], f16, tag="w2")
            for kk in range(2):
                nc.sync.dma_start(w2_t[:, kk, :], W2e[kk * 128:(kk + 1) * 128, :])
            w3_t = cpool.tile([128, 2, 64], f16, tag="w3")
            for kk in range(2):
                nc.sync.dma_start(w3_t[:, kk, :], W3e[kk * 128:(kk + 1) * 128, :])
            b1_t = cpool.tile([128, HC], f32, tag="b1")
            nc.sync.dma_start(b1_t[:], b1r[:])
            b2_t = cpool.tile([128, HC], f32, tag="b2")
            nc.sync.dma_start(b2_t[:], b2r[:])
            b3_t = cpool.tile([128, OUT], f32, tag="b3")
            nc.sync.dma_start(b3_t[:], b3r[:])
            c2_t = cpool.tile([128, 272], f32, tag="c2")
            nc.sync.dma_start(c2_t[:], c2r[:])
            c3_t = cpool.tile([128, 64], f32, tag="c3")
            nc.sync.dma_start(c3_t[:], c3r[:])

            hT_t = rpool.tile([128, 2, NB, 128], f16, tag="hT")
            ald1_t = rpool.tile([128, NB, HEADS], f16, tag="ald1")
            ald2_t = rpool.tile([128, NB, HEADS], f16, tag="ald2")
            ald3_t = rpool.tile([128, NB, 1], f16, tag="ald3")

            def shard_write(xe_sa, xe_sb, b, src_ap, wr):
                tgt = xe_sa if b < NBH else xe_sb
                r0 = (b % NBH) * 128
                nc.sync.dma_start(tgt[r0:r0 + 128, 0:wr], src_ap)

            def allgather(src_dram, dst_ap):
                nc.gpsimd.collective_compute(
                    "AllGather", mybir.AluOpType.bypass,
                    replica_groups=[list(range(NCORES))],
                    ins=[src_dram.ap().opt()], outs=[dst_ap.opt()])

            def phase_a1():
                # replicated: every core computes the full xe1 locally
                for t in range(NBLK_TOT):
                    c, b = divmod(t, NB)
                    row0 = (c * HR + b * 128 if b < NBH
                            else NCORES * HR + c * HR + (b - NBH) * 128)
                    lhs = spool.tile([128, 128], f16, tag="lhsA")
                    nc.sync.dma_start(lhs[:], xTf[:, t * 128:(t + 1) * 128])
                    ps = psA.tile([128, 272], f32, tag="psA")
                    nc.tensor.matmul(ps[:], lhs[:],
                                     w1_t[:], start=True, stop=True)
                    xa = ppool.tile([128, 264], f16, tag="xeA")
                    nc.vector.tensor_copy(xa[:], ps[:, 0:264])
                    nc.sync.dma_start(xe1f[row0:row0 + 128, 0:264], xa[:])
                # own blocks only: al_d1 columns
                for t in range(NB):
                    ps = psAD.tile([128, CK, HEADS], f32, tag="adps")
                    nc.tensor.matmul(ps[:, 0, :], xT_t[:, t * 128:(t + 1) * 128],
                                     w1_t[:, 264:272], start=True, stop=True)
                    nc.vector.tensor_copy(ald1_t[:, t, :], ps[:, 0, :])

            def phase_b(L, xe_f, ald_t, b_t, wn_t, corr_t, ald_next,
                        xe_nsa, xe_nsb):
                nh = HEADS if L < 3 else 1
                fe = HC if L < 3 else OUT
                row = ROW12 if L < 3 else ROW3
                for b in range(NB):
                    ofs = b * S
                    # transposed one-hot from host; derive ohd by PE transpose
                    oht = ohtpool.tile([128, CK, 128], f16, tag="oht")
                    nc.sync.dma_start(
                        oht[:].rearrange("p c q -> p (c q)"),
                        ohdT[:, ofs:ofs + S])
                    ohd = ohpool.tile([128, CK, 128], f16, tag="ohd")
                    for grp in range((CK + 3) // 4):
                        c0 = grp * 4
                        cw = min(4, CK - c0)
                        pt = psT.tile([128, 4, 128], f16, tag="tr4")
                        for j in range(cw):
                            nc.tensor.transpose(
                                pt[:, j, :], oht[:, c0 + j, :], ident_t[:])
                        nc.vector.tensor_copy(
                            ohd[:, c0:c0 + cw, :], pt[:, 0:cw, :])
                    # al_d per edge slot = ohdT^T @ al_d_blk  (TensorE)
                    adps = psAD.tile([128, CK, HEADS], f32, tag="adps")
                    for c in range(CK):
                        nc.tensor.matmul(adps[:, c, 0:nh], oht[:, c, :],
                                         ald_t[:, b, 0:nh],
                                         start=True, stop=True)
                    # THE gather: [xp | al_s] rows by src
                    g = (gpool if L < 3 else g3pool).tile(
                        [128, CK, row], f16, tag="g" if L < 3 else "g3")
                    SH = S // 2
                    CH = CK // 2
                    for p in range(2):
                        isl = gsrc_t[:, (b * S + p * SH) // 16:
                                     (b * S + (p + 1) * SH) // 16]
                        nc.gpsimd.dma_gather(
                            g[:, p * CH:(p + 1) * CH, :], xe_f[:, 0:row],
                            isl, SH, SH, row, elem_step=row,
                            single_packet=False,
                            queue_num=(2 * b + p) % NQ)
                    # ee = exp(lrelu(al_s + al_d))
                    z = spool.tile([128, CK, nh], f32, tag="z")
                    nc.vector.tensor_tensor(
                        z[:], g[:, :, fe:fe + nh], adps[:, :, 0:nh], Alu.add)
                    zf = z[:].rearrange("p c h -> p (c h)")
                    nc.vector.scalar_tensor_tensor(
                        zf, zf, NEG, zf, Alu.mult, Alu.max)
                    ee = spool.tile([128, CK, nh], f16, tag="ee")
                    nc.scalar.activation(
                        ee[:].rearrange("p c h -> p (c h)"), zf, Act.Exp)
                    # scale features by ee
                    if L < 3:
                        nc.vector.tensor_tensor(
                            g[:, :, 0:fe].rearrange("p c (h w) -> p c h w", w=HID),
                            g[:, :, 0:fe].rearrange("p c (h w) -> p c h w", w=HID),
                            ee[:].to_broadcast([128, CK, nh, HID]),
                            Alu.mult)
                    else:
                        nc.vector.tensor_tensor(
                            g[:, :, 0:fe], g[:, :, 0:fe],
                            ee[:].rearrange("p c h -> p (c h)").to_broadcast(
                                [128, CK, fe]),
                            Alu.mult)
                    # segment-sum + ee denominator via interleaved matmuls
                    ps = psB.tile([128, 264], f32, tag="agg")
                    for c in range(CK):
                        nc.tensor.matmul(
                            ps[:, 0:fe], ohd[:, c, :], g[:, c, 0:fe],
                            start=(c == 0), stop=(c == CK - 1))
                    for c in range(CK):
                        nc.tensor.matmul(
                            ps[:, fe:fe + nh], ohd[:, c, :], ee[:, c, :],
                            start=(c == 0), stop=(c == CK - 1))
                    # h = ps * 1/(s+eps) + bias
                    r = spool.tile([128, nh], f32, tag="r")
                    nc.vector.reciprocal(r[:], ps[:, fe:fe + nh])
                    h = ppool.tile([128, fe], f32, tag="h")
                    if L < 3:
                        nc.vector.tensor_tensor(
                            h[:].rearrange("p (x w) -> p x w", w=HID),
                            ps[:, 0:fe].rearrange("p (x w) -> p x w", w=HID),
                            r[:].to_broadcast([128, nh, HID]),
                            Alu.mult)
                        nc.vector.tensor_tensor(h[:], h[:], b_t[:], Alu.add)
                        # h' = elu(h)+1 = max(h,0) + exp(min(h,0))
                        t2 = ppool.tile([128, fe], f32, tag="elu")
                        nc.vector.scalar_tensor_tensor(
                            t2[:], h[:], 0.0, h[:], Alu.min, Alu.min)
                        nc.scalar.activation(t2[:], t2[:], Act.Exp)
                        h16 = ppool.tile([128, fe], f16, tag="h16")
                        nc.vector.scalar_tensor_tensor(
                            h16[:], h[:], 0.0, t2[:], Alu.max, Alu.add)
                        for half in range(2):
                            pt = psT.tile([128, 4, 128], f16, tag="tr4")
                            nc.tensor.transpose(
                                pt[:, 0, :], h16[:, half * 128:(half + 1) * 128],
                                ident_t[:])
                            nc.vector.tensor_copy(hT_t[:, half, b, :],
                                                  pt[:, 0, :])
                        # embedded next-layer phase A (h' @ W - colsum(W))
                        ncols = 272 if L == 1 else 64
                        wr = 264 if L == 1 else 41
                        nhn = 8 if L == 1 else 1
                        psa = psA.tile([128, 272], f32, tag="psA")
                        for kk in range(2):
                            nc.tensor.matmul(
                                psa[:, 0:ncols], hT_t[:, kk, b, :],
                                wn_t[:, kk, 0:ncols],
                                start=(kk == 0), stop=(kk == 1))
                        xa = ppool.tile([128, 264], f16, tag="xeA")
                        nc.vector.tensor_tensor(
                            xa[:, 0:wr], psa[:, 0:wr], corr_t[:, 0:wr],
                            Alu.subtract)
                        nc.vector.tensor_tensor(
                            ald_next[:, b, 0:nhn], psa[:, wr:wr + nhn],
                            corr_t[:, wr:wr + nhn], Alu.subtract)
                        shard_write(xe_nsa, xe_nsb, b, xa[:, 0:wr], wr)
                        if b == NBH - 1:
                            allgather(xe_nsa,
                                      (xe2f if L == 1 else xe3f)[0:NCORES * HR, :])
                        elif b == NB - 1:
                            allgather(xe_nsb,
                                      (xe2f if L == 1 else xe3f)[NCORES * HR:NPAD, :])
                    else:
                        # log_softmax over the 40 classes
                        nc.vector.tensor_tensor(
                            h[:], ps[:, 0:fe],
                            r[:].to_broadcast([128, fe]), Alu.mult)
                        nc.vector.tensor_tensor(h[:], h[:], b_t[:], Alu.add)
                        m_t = spool.tile([128, 1], f32, tag="m")
                        nc.vector.tensor_reduce(
                            m_t[:], h[:], mybir.AxisListType.X, Alu.max)
                        nc.vector.tensor_tensor(
                            h[:], h[:], m_t[:].to_broadcast([128, fe]),
                            Alu.subtract)
                        x_t = ppool.tile([128, fe], f32, tag="exps")
                        s_t = spool.tile([128, 1], f32, tag="s")
                        nc.scalar.activation(
                            x_t[:], h[:], Act.Exp, accum_out=s_t[:])
                        l_t = spool.tile([128, 1], f32, tag="l")
                        nc.scalar.activation(l_t[:], s_t[:], Act.Ln)
                        nc.vector.tensor_tensor(
                            h[:], h[:], l_t[:].to_broadcast([128, fe]),
                            Alu.subtract)
                        nc.sync.dma_start(out[b * 128:(b + 1) * 128, :], h[:])

            phase_a1()
            phase_b(1, xe1f, ald1_t, b1_t, w2_t, c2_t, ald2_t, xe2sa, xe2sb)
            phase_b(2, xe2f, ald2_t, b2_t, w3_t, c3_t, ald3_t, xe3sa, xe3sb)
            phase_b(3, xe3f, ald3_t, b3_t, None, None, None, None, None)

    nc.compile()
    return nc


# ----------------------------------------------------------------------------
# entry point
# ----------------------------------------------------------------------------

LAST_EXEC_NS = None


def kernel(**inputs):
    from concourse.bass_utils import run_bass_kernel_spmd
    global LAST_EXEC_NS

    x = np.asarray(inputs["x"], np.float32)
    ei = np.asarray(inputs["edge_index"])
    CK, cores = preprocess(ei)

    xT16 = np.zeros((IN, NPAD), np.float16)
    xT16[:, 0:N] = x.T.astype(np.float16)
    W1en = build_w_ext(np.asarray(inputs["W1"], np.float32),
                       np.asarray(inputs["a_src1"], np.float32),
                       np.asarray(inputs["a_dst1"], np.float32), 272)
    W2en = build_w_ext(np.asarray(inputs["W2"], np.float32),
                       np.asarray(inputs["a_src2"], np.float32),
                       np.asarray(inputs["a_dst2"], np.float32), 272)
    W3_ = np.asarray(inputs["W3"], np.float32)
    W3en = np.zeros((HC, 64), np.float32)
    W3en[:, 0:OUT] = W3_
    W3en[:, OUT:OUT + 1] = W3_ @ np.asarray(inputs["a_src3"], np.float32).reshape(OUT, 1)
    W3en[:, OUT + 1:OUT + 2] = W3_ @ np.asarray(inputs["a_dst3"], np.float32).reshape(OUT, 1)
    W3en = W3en.astype(np.float16)

    # ELU -1 fold: colsum correction rows for the next layer's W_ext
    c2n = np.tile(W2en.astype(np.float32).sum(axis=0), (128, 1)).astype(np.float32)
    c3n = np.tile(W3en.astype(np.float32).sum(axis=0), (128, 1)).astype(np.float32)

    ident_n = np.eye(128, dtype=np.float16)
    b1n = np.tile(np.asarray(inputs["b1"], np.float32), (128, 1))
    b2n = np.tile(np.asarray(inputs["b2"], np.float32), (128, 1))
    b3n = np.tile(np.asarray(inputs["b3"], np.float32), (128, 1))

    nc = build_nc(CK)
    in_maps = []
    for k in range(NCORES):
        in_maps.append({
            "xT16": np.ascontiguousarray(xT16[:, k * NB * 128:(k + 1) * NB * 128]),
            "xTf": xT16,
            "W1e": W1en, "W2e": W2en, "W3e": W3en,
            "gsrc": cores[k]["gsrc"], "ohdT": cores[k]["ohdT"],
            "ident": ident_n,
            "b1r": b1n, "b2r": b2n, "b3r": b3n, "c2r": c2n, "c3r": c3n,
        })
    trace = bool(int(os.environ.get("GAT_TRACE", "0")))
    res = run_bass_kernel_spmd(nc, in_maps, list(range(NCORES)), trace=trace)
    LAST_EXEC_NS = res.exec_time_ns
    full = np.concatenate([res.results[k]["out"] for k in range(NCORES)], axis=0)
    return full[0:N].astype(np.float32)


# revision 12
# speedup vs baseline: 3.9410x; 1.0704x over previous
"""Trainium2 Bass kernel for 3-layer GAT (nn_MultiLayerGAT), v3.

Strategy (dst-node sharding, 8 cores, fp16 data path):
  - Add self-loops, sort edges by dst. Nodes padded to 10240 = 80 blocks of
    128; core k owns blocks [10k, 10k+10). Each block's edges padded to CK
    chunks of 128 slots (same CK for all cores => one SPMD program).
  - Phase A is SHARDED: each core computes xp_ext = h @ W_ext for its own 10
    blocks only, writing two [640, row] fp16 half-shards; two AllGathers
    (issued as soon as each half is complete, overlapping phase B of the
    previous layer) replicate the full xe to every core.  xe row =
    [xp | al_s | pad] fp16; al_d stays in a per-core resident SBUF tile.
    xe_full row layout is region-major: node (c, b, p) lives at row
    c*640 + b*128 + p for blocks b<5, and 5120 + c*640 + (b-5)*128 + p for
    b>=5 (so each AllGather writes one contiguous region); gather indices
    are host-remapped accordingly.
  - Phase B per dst block:
      The TRANSPOSED dst one-hot (ohdT[q, slot] = dstcode[slot]==q) is
      host-precomputed and DMA-streamed; the aggregation one-hot ohd is
      derived on-chip by PE transposes (batched PSUM->SBUF copies).
      al_d per edge slot = ohdT^T @ al_d_blk on TensorE (no dma_gather).
      ONE gpsimd dma_gather per block fetches [xp|al_s] rows by src (768B
      fp16 elements).  Gathers rotate across 4 SWDGE queues so descriptor
      generation runs concurrently on different Q7 core pairs (desc-gen at
      ~41us/4352 idxs is otherwise the serial critical path; measured 2.5x
      faster with 4 queues).
      ee = exp(lrelu(al_s + al_d)); scale gathered xp by ee; segment-sum
      AND the ee denominator via interleaved one-hot matmuls on TensorE
      (fp16, fp32 PSUM): ps[:,0:fe] += ohd_c^T @ (ee*G)_c and
      ps[:,fe:fe+nh] += ohd_c^T @ ee_c.
      h = ps*recip(s+eps) + bias; layers 1-2 store h' = elu(h)+1 = max(h,0)
      + exp(min(h,0)) (the -1 is folded into the next layer's phase A as a
      host-precomputed column-sum correction row), transpose into the
      resident h^T, and run the next layer's phase A for this block
      immediately.
  - Final layer: single head, fp16 rows of 128 cols, log_softmax per block.

Pads: gather idx 0 (finite), dstcode -1 => one-hot column all zero, so pads
contribute nothing to numerator or denominator.
"""

import os
import numpy as np

N = 10000
E = 320000
IN = 128
HID = 32
HEADS = 8
HC = HEADS * HID          # 256
OUT = 40
NEG = 0.2

NPAD = 10240              # 80 blocks of 128
NBLK_TOT = NPAD // 128    # 80
NCORES = 8
NB = NBLK_TOT // NCORES   # 10 blocks per core
NBH = NB // 2             # blocks per half-shard

ROW12 = 384               # xe row fp16 cols, layers 1-2: [xp 256 | al_s 8 | pad]
ROW3 = 128                # layer 3: [xp 40 | al_s 1 | pad]

NQ = int(os.environ.get("GAT_QUEUES", "4"))      # SWDGE queues for gathers
GBUFS = int(os.environ.get("GAT_GBUFS", "4"))    # gather tiles in flight


def rowmap(node):
    """xe_full row of a global (padded) node id, region-major halves."""
    c, r = np.divmod(node, NB * 128)
    b, p = np.divmod(r, 128)
    lo = b < NBH
    return np.where(lo,
                    c * (NBH * 128) + b * 128 + p,
                    NCORES * NBH * 128 + c * (NBH * 128) + (b - NBH) * 128 + p)


# ----------------------------------------------------------------------------
# host-side preprocessing
# ----------------------------------------------------------------------------

def build_w_ext(W, a_src, a_dst, row):
    """W_ext[in, cols]: [W | W@As | W@Ad] (block-diag attention vectors)."""
    inn, hc = W.shape
    H, C = a_src.shape
    As = np.zeros((hc, H), np.float32)
    Ad = np.zeros((hc, H), np.float32)
    for h in range(H):
        As[h * C:(h + 1) * C, h] = a_src[h]
        Ad[h * C:(h + 1) * C, h] = a_dst[h]
    We = np.zeros((inn, row), np.float32)
    We[:, 0:hc] = W
    We[:, hc:hc + H] = W @ As
    We[:, hc + H:hc + 2 * H] = W @ Ad
    return We.astype(np.float16)


def preprocess(edge_index):
    """Chunk tables shared by all layers. Returns (CK, per-core arrays)."""
    src = np.concatenate([edge_index[0], np.arange(N, dtype=edge_index.dtype)])
    dst = np.concatenate([edge_index[1], np.arange(N, dtype=edge_index.dtype)])
    src = src.astype(np.int64)
    dst = dst.astype(np.int64)
    order = np.argsort(dst, kind="stable")
    ssrc, sdst = src[order], dst[order]
    blk = sdst // 128
    cnt = np.bincount(blk, minlength=NBLK_TOT)
    CK = int(np.ceil(cnt.max() / 128))
    S = CK * 128
    starts = np.concatenate([[0], np.cumsum(cnt)])

    gsrc = np.zeros((NBLK_TOT, S), np.int64)           # gather row (by src)
    dstc = np.full((NBLK_TOT, S), -1, np.int32)        # dst - 128*block
    srows = rowmap(ssrc)
    for b in range(NBLK_TOT):
        lo, hi = starts[b], starts[b + 1]
        n = hi - lo
        gsrc[b, :n] = srows[lo:hi]
        dstc[b, :n] = (sdst[lo:hi] - 128 * b).astype(np.int32)

    def wrap16(idx_flat):
        t16 = idx_flat.reshape(S // 16, 16).T.astype(np.int16)
        return np.tile(t16, (8, 1))

    qvec = np.arange(128, dtype=np.int32).reshape(128, 1)
    cores = []
    for k in range(NCORES):
        bs = range(k * NB, (k + 1) * NB)
        gsrc_t = np.concatenate([wrap16(gsrc[b]) for b in bs], axis=1)
        # ohdT[q, (b*CK + c)*128 + p] = (dstc[b, c*128+p] == q), fp16
        codes = dstc[k * NB:(k + 1) * NB].reshape(1, NB * S)
        ohdT = (qvec == codes).astype(np.float16)
        cores.append(dict(gsrc=gsrc_t, ohdT=np.ascontiguousarray(ohdT)))
    return CK, cores


# ----------------------------------------------------------------------------
# bass program
# ----------------------------------------------------------------------------

def build_nc(CK):
    import concourse.bacc as bacc
    import concourse.mybir as mybir
    import concourse.tile as tile
    from concourse.library_config import mlp

    f32 = mybir.dt.float32
    f16 = mybir.dt.float16
    i16 = mybir.dt.int16
    Alu = mybir.AluOpType
    Act = mybir.ActivationFunctionType

    S = CK * 128
    HR = NBH * 128            # rows per half-shard (640)

    nc = bacc.Bacc("TRN2", debug=False, num_swdge_queues=NQ)

    # inputs (per core)
    xT16 = nc.dram_tensor("xT16", [IN, NB * 128], f16, kind="ExternalInput")
    xTf = nc.dram_tensor("xTf", [IN, NPAD], f16, kind="ExternalInput")
    W1e = nc.dram_tensor("W1e", [IN, 272], f16, kind="ExternalInput")
    W2e = nc.dram_tensor("W2e", [HC, 272], f16, kind="ExternalInput")
    W3e = nc.dram_tensor("W3e", [HC, 64], f16, kind="ExternalInput")
    gsrc = nc.dram_tensor("gsrc", [128, NB * S // 16], i16, kind="ExternalInput")
    ohdT = nc.dram_tensor("ohdT", [128, NB * S], f16, kind="ExternalInput")
    ident = nc.dram_tensor("ident", [128, 128], f16, kind="ExternalInput")
    b1r = nc.dram_tensor("b1r", [128, HC], f32, kind="ExternalInput")
    b2r = nc.dram_tensor("b2r", [128, HC], f32, kind="ExternalInput")
    b3r = nc.dram_tensor("b3r", [128, OUT], f32, kind="ExternalInput")
    c2r = nc.dram_tensor("c2r", [128, 272], f32, kind="ExternalInput")
    c3r = nc.dram_tensor("c3r", [128, 64], f32, kind="ExternalInput")

    out = nc.dram_tensor("out", [NB * 128, OUT], f32, kind="ExternalOutput")

    # scratch DRAM: two half-shards + two-region full tensors per layer
    def xe_pair(name, row):
        sa = nc.dram_tensor(name + "sa", [HR, row], f16)
        sb = nc.dram_tensor(name + "sb", [HR, row], f16)
        ff = nc.dram_tensor(name + "f", [NPAD, row], f16, addr_space="Shared")
        return sa, sb, ff

    xe1f = nc.dram_tensor("xe1f", [NPAD, ROW12], f16)
    xe2sa, xe2sb, xe2f = xe_pair("xe2", ROW12)
    xe3sa, xe3sb, xe3f = xe_pair("xe3", ROW3)

    with tile.TileContext(nc) as tc:
        nc.gpsimd.load_library(mlp)
        with tc.tile_pool(name="const", bufs=1) as cpool, \
             tc.tile_pool(name="res", bufs=1) as rpool, \
             tc.tile_pool(name="oht", bufs=3) as ohtpool, \
             tc.tile_pool(name="ohd", bufs=2) as ohpool, \
             tc.tile_pool(name="g", bufs=GBUFS) as gpool, \
             tc.tile_pool(name="g3", bufs=3) as g3pool, \
             tc.tile_pool(name="small", bufs=2) as spool, \
             tc.tile_pool(name="post", bufs=2) as ppool, \
             tc.tile_pool(name="psA", bufs=2, space="PSUM") as psA, \
             tc.tile_pool(name="psB", bufs=2, space="PSUM") as psB, \
             tc.tile_pool(name="psT", bufs=2, space="PSUM") as psT, \
             tc.tile_pool(name="psAD", bufs=2, space="PSUM") as psAD:

            gsrc_t = cpool.tile([128, NB * S // 16], i16, tag="gsrc")
            nc.sync.dma_start(gsrc_t[:], gsrc[:])
            ident_t = cpool.tile([128, 128], f16, tag="ident")
            nc.sync.dma_start(ident_t[:], ident[:])
            xT_t = cpool.tile([128, NB * 128], f16, tag="xT")
            nc.sync.dma_start(xT_t[:], xT16[:])

            w1_t = cpool.tile([128, 272], f16, tag="w1")
            nc.sync.dma_start(w1_t[:], W1e[:])
            w2_t = cpool.tile([128, 2, 272], f16, tag="w2")
            for kk in range(2):
                nc.sync.dma_start(w2_t[:, kk, :], W2e[kk * 128:(kk + 1) * 128, :])
            w3_t = cpool.tile([128, 2, 64], f16, tag="w3")
            for kk in range(2):
                nc.sync.dma_start(w3_t[:, kk, :], W3e[kk * 128:(kk + 1) * 128, :])
            b1_t = cpool.tile([128, HC], f32, tag="b1")
            nc.sync.dma_start(b1_t[:], b1r[:])
            b2_t = cpool.tile([128, HC], f32, tag="b2")
            nc.sync.dma_start(b2_t[:], b2r[:])
            b3_t = cpool.tile([128, OUT], f32, tag="b3")
            nc.sync.dma_start(b3_t[:], b3r[:])
            c2_t = cpool.tile([128, 272], f32, tag="c2")
            nc.sync.dma_start(c2_t[:], c2r[:])
            c3_t = cpool.tile([128, 64], f32, tag="c3")
            nc.sync.dma_start(c3_t[:], c3r[:])

            hT_t = rpool.tile([128, 2, NB, 128], f16, tag="hT")
            ald1_t = rpool.tile([128, NB, HEADS], f16, tag="ald1")
            ald2_t = rpool.tile([128, NB, HEADS], f16, tag="ald2")
            ald3_t = rpool.tile([128, NB, 1], f16, tag="ald3")

            def shard_write(xe_sa, xe_sb, b, src_ap, wr):
                tgt = xe_sa if b < NBH else xe_sb
                r0 = (b % NBH) * 128
                nc.sync.dma_start(tgt[r0:r0 + 128, 0:wr], src_ap)

            def allgather(src_dram, dst_ap):
                nc.gpsimd.collective_compute(
                    "AllGather", mybir.AluOpType.bypass,
                    replica_groups=[list(range(NCORES))],
                    ins=[src_dram.ap().opt()], outs=[dst_ap.opt()])

            def phase_a1():
                # replicated: every core computes the full xe1 locally
                for t in range(NBLK_TOT):
                    c, b = divmod(t, NB)
                    row0 = (c * HR + b * 128 if b < NBH
                            else NCORES * HR + c * HR + (b - NBH) * 128)
                    lhs = spool.tile([128, 128], f16, tag="lhsA")
                    nc.sync.dma_start(lhs[:], xTf[:, t * 128:(t + 1) * 128])
                    ps = psA.tile([128, 272], f32, tag="psA")
                    nc.tensor.matmul(ps[:], lhs[:],
                                     w1_t[:], start=True, stop=True)
                    xa = ppool.tile([128, 264], f16, tag="xeA")
                    nc.scalar.activation(xa[:], ps[:, 0:264], Act.Copy)
                    nc.sync.dma_start(xe1f[row0:row0 + 128, 0:264], xa[:])
                # own blocks only: al_d1 columns
                for t in range(NB):
                    ps = psAD.tile([128, CK, HEADS], f32, tag="adps")
                    nc.tensor.matmul(ps[:, 0, :], xT_t[:, t * 128:(t + 1) * 128],
                                     w1_t[:, 264:272], start=True, stop=True)
                    nc.vector.tensor_copy(ald1_t[:, t, :], ps[:, 0, :])

            def phase_b(L, xe_f, ald_t, b_t, wn_t, corr_t, ald_next,
                        xe_nsa, xe_nsb):
                nh = HEADS if L < 3 else 1
                fe = HC if L < 3 else OUT
                row = ROW12 if L < 3 else ROW3
                for b in range(NB):
                    ofs = b * S
                    # transposed one-hot from host; derive ohd by PE transpose
                    oht = ohtpool.tile([128, CK, 128], f16, tag="oht")
                    nc.sync.dma_start(
                        oht[:].rearrange("p c q -> p (c q)"),
                        ohdT[:, ofs:ofs + S])
                    ohd = ohpool.tile([128, CK, 128], f16, tag="ohd")
                    for grp in range((CK + 3) // 4):
                        c0 = grp * 4
                        cw = min(4, CK - c0)
                        pt = psT.tile([128, 4, 128], f16, tag="tr4")
                        for j in range(cw):
                            nc.tensor.transpose(
                                pt[:, j, :], oht[:, c0 + j, :], ident_t[:])
                        nc.vector.tensor_copy(
                            ohd[:, c0:c0 + cw, :], pt[:, 0:cw, :])
                    # al_d per edge slot = ohdT^T @ al_d_blk  (TensorE)
                    adps = psAD.tile([128, CK, HEADS], f32, tag="adps")
                    for c in range(CK):
                        nc.tensor.matmul(adps[:, c, 0:nh], oht[:, c, :],
                                         ald_t[:, b, 0:nh],
                                         start=True, stop=True)
                    # THE gather: [xp | al_s] rows by src
                    g = (gpool if L < 3 else g3pool).tile(
                        [128, CK, row], f16, tag="g" if L < 3 else "g3")
                    NP = min(4, NQ)
                    cuts = [round(p * CK / NP) for p in range(NP + 1)]
                    for p in range(NP):
                        c0, c1 = cuts[p], cuts[p + 1]
                        sh = (c1 - c0) * 128
                        isl = gsrc_t[:, (b * S + c0 * 128) // 16:
                                     (b * S + c1 * 128) // 16]
                        nc.gpsimd.dma_gather(
                            g[:, c0:c1, :], xe_f[:, 0:row],
                            isl, sh, sh, row, elem_step=row,
                            single_packet=False,
                            queue_num=(NP * b + p) % NQ)
                    # ee = exp(lrelu(al_s + al_d))
                    z = spool.tile([128, CK, nh], f32, tag="z")
                    nc.vector.tensor_tensor(
                        z[:], g[:, :, fe:fe + nh], adps[:, :, 0:nh], Alu.add)
                    zf = z[:].rearrange("p c h -> p (c h)")
                    nc.vector.scalar_tensor_tensor(
                        zf, zf, NEG, zf, Alu.mult, Alu.max)
                    ee = spool.tile([128, CK, nh], f16, tag="ee")
                    nc.scalar.activation(
                        ee[:].rearrange("p c h -> p (c h)"), zf, Act.Exp)
                    # scale features by ee
                    if L < 3:
                        nc.vector.tensor_tensor(
                            g[:, :, 0:fe].rearrange("p c (h w) -> p c h w", w=HID),
                            g[:, :, 0:fe].rearrange("p c (h w) -> p c h w", w=HID),
                            ee[:].to_broadcast([128, CK, nh, HID]),
                            Alu.mult)
                    else:
                        nc.vector.tensor_tensor(
                            g[:, :, 0:fe], g[:, :, 0:fe],
                            ee[:].rearrange("p c h -> p (c h)").to_broadcast(
                                [128, CK, fe]),
                            Alu.mult)
                    # segment-sum + ee denominator via interleaved matmuls
                    ps = psB.tile([128, 264], f32, tag="agg")
                    for c in range(CK):
                        nc.tensor.matmul(
                            ps[:, 0:fe], ohd[:, c, :], g[:, c, 0:fe],
                            start=(c == 0), stop=(c == CK - 1))
                    for c in range(CK):
                        nc.tensor.matmul(
                            ps[:, fe:fe + nh], ohd[:, c, :], ee[:, c, :],
                            start=(c == 0), stop=(c == CK - 1))
                    # h = ps * 1/(s+eps) + bias
                    r = spool.tile([128, nh], f32, tag="r")
                    nc.vector.reciprocal(r[:], ps[:, fe:fe + nh])
                    h = ppool.tile([128, fe], f32, tag="h")
                    if L < 3:
                        nc.vector.tensor_tensor(
                            h[:].rearrange("p (x w) -> p x w", w=HID),
                            ps[:, 0:fe].rearrange("p (x w) -> p x w", w=HID),
                            r[:].to_broadcast([128, nh, HID]),
                            Alu.mult)
                        nc.vector.tensor_tensor(h[:], h[:], b_t[:], Alu.add)
                        # h' = elu(h)+1 = max(h,0) + exp(min(h,0))
                        t2 = ppool.tile([128, fe], f32, tag="elu")
                        nc.vector.scalar_tensor_tensor(
                            t2[:], h[:], 0.0, h[:], Alu.min, Alu.min)
                        nc.scalar.activation(t2[:], t2[:], Act.Exp)
                        h16 = ppool.tile([128, fe], f16, tag="h16")
                        nc.vector.scalar_tensor_tensor(
                            h16[:], h[:], 0.0, t2[:], Alu.max, Alu.add)
                        for half in range(2):
                            pt = psT.tile([128, 4, 128], f16, tag="tr4")
                            nc.tensor.transpose(
                                pt[:, 0, :], h16[:, half * 128:(half + 1) * 128],
                                ident_t[:])
                            nc.vector.tensor_copy(hT_t[:, half, b, :],
                                                  pt[:, 0, :])
                        # embedded next-layer phase A (h' @ W - colsum(W))
                        ncols = 272 if L == 1 else 64
                        wr = 264 if L == 1 else 41
                        nhn = 8 if L == 1 else 1
                        psa = psA.tile([128, 272], f32, tag="psA")
                        for kk in range(2):
                            nc.tensor.matmul(
                                psa[:, 0:ncols], hT_t[:, kk, b, :],
                                wn_t[:, kk, 0:ncols],
                                start=(kk == 0), stop=(kk == 1))
                        xa = ppool.tile([128, 264], f16, tag="xeA")
                        nc.vector.tensor_tensor(
                            xa[:, 0:wr], psa[:, 0:wr], corr_t[:, 0:wr],
                            Alu.subtract)
                        nc.vector.tensor_tensor(
                            ald_next[:, b, 0:nhn], psa[:, wr:wr + nhn],
                            corr_t[:, wr:wr + nhn], Alu.subtract)
                        shard_write(xe_nsa, xe_nsb, b, xa[:, 0:wr], wr)
                        if b == NBH - 1:
                            allgather(xe_nsa,
                                      (xe2f if L == 1 else xe3f)[0:NCORES * HR, :])
                        elif b == NB - 1:
                            allgather(xe_nsb,
                                      (xe2f if L == 1 else xe3f)[NCORES * HR:NPAD, :])
                    else:
                        # log_softmax over the 40 classes
                        nc.vector.tensor_tensor(
                            h[:], ps[:, 0:fe],
                            r[:].to_broadcast([128, fe]), Alu.mult)
                        nc.vector.tensor_tensor(h[:], h[:], b_t[:], Alu.add)
                        m_t = spool.tile([128, 1], f32, tag="m")
                        nc.vector.tensor_reduce(
                            m_t[:], h[:], mybir.AxisListType.X, Alu.max)
                        nc.vector.tensor_tensor(
                            h[:], h[:], m_t[:].to_broadcast([128, fe]),
                            Alu.subtract)
                        x_t = ppool.tile([128, fe], f32, tag="exps")
                        s_t = spool.tile([128, 1], f32, tag="s")
                        nc.scalar.activation(
                            x_t[:], h[:], Act.Exp, accum_out=s_t[:])
                        l_t = spool.tile([128, 1], f32, tag="l")
                        nc.scalar.activation(l_t[:], s_t[:], Act.Ln)
                        nc.vector.tensor_tensor(
                            h[:], h[:], l_t[:].to_broadcast([128, fe]),
                            Alu.subtract)
                        nc.sync.dma_start(out[b * 128:(b + 1) * 128, :], h[:])

            phase_a1()
            phase_b(1, xe1f, ald1_t, b1_t, w2_t, c2_t, ald2_t, xe2sa, xe2sb)
            phase_b(2, xe2f, ald2_t, b2_t, w3_t, c3_t, ald3_t, xe3sa, xe3sb)
            phase_b(3, xe3f, ald3_t, b3_t, None, None, None, None, None)

    nc.compile()
    return nc


# ----------------------------------------------------------------------------
# entry point
# ----------------------------------------------------------------------------

LAST_EXEC_NS = None


def kernel(**inputs):
    from concourse.bass_utils import run_bass_kernel_spmd
    global LAST_EXEC_NS

    x = np.asarray(inputs["x"], np.float32)
    ei = np.asarray(inputs["edge_index"])
    CK, cores = preprocess(ei)

    xT16 = np.zeros((IN, NPAD), np.float16)
    xT16[:, 0:N] = x.T.astype(np.float16)
    W1en = build_w_ext(np.asarray(inputs["W1"], np.float32),
                       np.asarray(inputs["a_src1"], np.float32),
                       np.asarray(inputs["a_dst1"], np.float32), 272)
    W2en = build_w_ext(np.asarray(inputs["W2"], np.float32),
                       np.asarray(inputs["a_src2"], np.float32),
                       np.asarray(inputs["a_dst2"], np.float32), 272)
    W3_ = np.asarray(inputs["W3"], np.float32)
    W3en = np.zeros((HC, 64), np.float32)
    W3en[:, 0:OUT] = W3_
    W3en[:, OUT:OUT + 1] = W3_ @ np.asarray(inputs["a_src3"], np.float32).reshape(OUT, 1)
    W3en[:, OUT + 1:OUT + 2] = W3_ @ np.asarray(inputs["a_dst3"], np.float32).reshape(OUT, 1)
    W3en = W3en.astype(np.float16)

    # ELU -1 fold: colsum correction rows for the next layer's W_ext
    c2n = np.tile(W2en.astype(np.float32).sum(axis=0), (128, 1)).astype(np.float32)
    c3n = np.tile(W3en.astype(np.float32).sum(axis=0), (128, 1)).astype(np.float32)

    ident_n = np.eye(128, dtype=np.float16)
    b1n = np.tile(np.asarray(inputs["b1"], np.float32), (128, 1))
    b2n = np.tile(np.asarray(inputs["b2"], np.float32), (128, 1))
    b3n = np.tile(np.asarray(inputs["b3"], np.float32), (128, 1))

    nc = build_nc(CK)
    in_maps = []
    for k in range(NCORES):
        in_maps.append({
            "xT16": np.ascontiguousarray(xT16[:, k * NB * 128:(k + 1) * NB * 128]),
            "xTf": xT16,
            "W1e": W1en, "W2e": W2en, "W3e": W3en,
            "gsrc": cores[k]["gsrc"], "ohdT": cores[k]["ohdT"],
            "ident": ident_n,
            "b1r": b1n, "b2r": b2n, "b3r": b3n, "c2r": c2n, "c3r": c3n,
        })
    trace = bool(int(os.environ.get("GAT_TRACE", "0")))
    res = run_bass_kernel_spmd(nc, in_maps, list(range(NCORES)), trace=trace)
    LAST_EXEC_NS = res.exec_time_ns
    full = np.concatenate([res.results[k]["out"] for k in range(NCORES)], axis=0)
    return full[0:N].astype(np.float32)
